# revision 1
# baseline (speedup 1.0000x reference)
"""Trainium2 Bass kernel for 3-level hierarchical hypergraph GNN (HGNN).

Strategy (8 NeuronCores, one SPMD NEFF, per-core index data):
  - Nodes of every level sharded round-robin: global id g -> core g%8, slot g//8.
  - Incidence entries assigned to the owner core of their node endpoint.
  - L_apply = two segment-sum passes:
      stage1 (edges): partial edge sums via dma_gather(node rows) + weighted
                      one-hot matmul into 128-row PSUM edge windows; AllReduce.
      stage2 (nodes): gather full-edge-table rows + one-hot matmul into local
                      node windows (complete rows, no reduction needed).
    Degree scalings (Dv^-1/2, De^-1) folded into per-entry weights.
  - Pools: same segment-sum, targets = remapped cluster rows; ReduceScatter
    leaves each core exactly its local cluster slice.
  - Unpool: dma_gather from AllGathered coarse tables.
  - Linears: per-128-row-chunk PE transpose + matmul; concat = two
    accumulating matmuls; bias via broadcast-tile add.
"""
import sys

sys.path.insert(0, "/opt/trn_rl_repo")
import numpy as np

C = 8
CH = 32  # gather chunk size in 128-entry blocks

N0, N1, N2 = 100000, 25000, 6250
E0, E1, E2 = 20000, 5000, 1250
D_IN, D_H, D_OUT = 128, 128, 64


def _pad128(n):
    return ((n + 127) // 128) * 128


def _pad_local(n):
    return _pad128(-(-n // C))


# ---------------------------------------------------------------- host side
def _degree_weights(vi, ei, n, e):
    ones = np.ones(len(vi), np.float32)
    dV = np.bincount(vi, weights=ones, minlength=n)
    dE = np.bincount(ei, weights=ones, minlength=e)
    dv_is = np.where(dV > 0, dV ** -0.5, 0.0).astype(np.float32)
    de_i = np.where(dE > 0, 1.0 / dE, 0.0).astype(np.float32)
    return dv_is, de_i


def _plane_idx(idx):
    """int array (L,) -> [128, L//16] int16 (16-partition wrap, replicated x8)."""
    assert len(idx) % 16 == 0
    assert idx.max(initial=0) < 32768
    return np.tile(idx.astype(np.int16).reshape(-1, 16).T, (C, 1)).copy()


def _plane_tw(tgt, wgt):
    """-> [128, 2*B] f32, columns (2b, 2b+1) = (target, weight) of block b."""
    nb = len(tgt) // 128
    out = np.empty((128, 2 * nb), np.float32)
    out[:, 0::2] = tgt.astype(np.float32).reshape(nb, 128).T
    out[:, 1::2] = wgt.astype(np.float32).reshape(nb, 128).T
    return out


class SegStage:
    """Host data for one segment-sum stage, uniform structure across cores."""

    def __init__(self, name, gidx, trow, wgt, n_rows_padded):
        self.name = name
        self.nw = n_rows_padded // 128
        cnts = np.stack([
            np.bincount(trow[c] // 128, minlength=self.nw) for c in range(C)
        ])
        self.bpw = np.maximum(1, -(-cnts.max(axis=0) // 128)).astype(np.int64)
        self.nblocks = int(self.bpw.sum())
        L = 128 * self.nblocks
        self.L = L
        self.idx_planes, self.tw_planes = [], []
        starts = np.concatenate([[0], np.cumsum(self.bpw[:-1])]) * 128
        for c in range(C):
            order = np.argsort(trow[c] // 128, kind="stable")
            gi = np.zeros(L, np.int64)
            tg = np.zeros(L, np.int64)
            wg = np.zeros(L, np.float32)
            w_of = trow[c] // 128
            pos = np.searchsorted(w_of[order], np.arange(self.nw))
            end = np.searchsorted(w_of[order], np.arange(self.nw), side="right")
            for w in range(self.nw):
                sel = order[pos[w]:end[w]]
                s = starts[w]
                gi[s:s + len(sel)] = gidx[c][sel]
                tg[s:s + len(sel)] = trow[c][sel] % 128
                wg[s:s + len(sel)] = wgt[c][sel]
            self.idx_planes.append(_plane_idx(gi))
            self.tw_planes.append(_plane_tw(tg, wg))

    def renamed(self, name):
        st = SegStage.__new__(SegStage)
        st.__dict__ = dict(self.__dict__)
        st.name = name
        return st


def _gather_planes(idx_per_core):
    """Plain gather streams (no reduction), padded to a 128 multiple."""
    L = _pad128(len(idx_per_core[0]))
    planes = []
    for c in range(C):
        gi = np.zeros(L, np.int64)
        gi[: len(idx_per_core[c])] = idx_per_core[c]
        planes.append(_plane_idx(gi))
    return L, planes


# ---------------------------------------------------------------- device side
class Builder:
    def __init__(self, nc, mybir):
        self.nc = nc
        self.mybir = mybir
        self.tc = None
        self.inputs = {}  # name -> per-core list of arrays (or one shared array)

    def add_input(self, name, shape, dtype, arrays):
        assert name not in self.inputs, name
        t = self.nc.dram_tensor(name, list(shape), dtype, kind="ExternalInput")
        self.inputs[name] = arrays
        return t

    def setup_pools(self, ctx):
        tc = self.tc
        self.p_const = ctx.enter_context(tc.tile_pool(name="const", bufs=1))
        self.p_gath = ctx.enter_context(tc.tile_pool(name="gath", bufs=3))
        self.p_meta = ctx.enter_context(tc.tile_pool(name="meta", bufs=3))
        self.p_oh = ctx.enter_context(tc.tile_pool(name="oh", bufs=4))
        self.p_fl = ctx.enter_context(tc.tile_pool(name="fl", bufs=4))
        self.p_lin = ctx.enter_context(tc.tile_pool(name="lin", bufs=3))
        self.p_ps = ctx.enter_context(tc.tile_pool(name="ps", bufs=2, space="PSUM"))
        self.p_ps2 = ctx.enter_context(tc.tile_pool(name="ps2", bufs=2, space="PSUM"))

    def setup_consts(self):
        f32 = self.mybir.dt.float32
        iota = np.tile(np.arange(128, dtype=np.float32), (128, 1))
        ident = np.eye(128, dtype=np.float32)
        self.iota_t = self.const_mat("c_iota", iota)
        self.ident_t = self.const_mat("c_ident", ident)

    def const_mat(self, name, arr):
        f32 = self.mybir.dt.float32
        arr = np.ascontiguousarray(arr, np.float32)
        d = self.add_input(name, list(arr.shape), f32, arr)
        t = self.p_const.tile(list(arr.shape), f32, tag=name)
        self.nc.sync.dma_start(t[:], d[:, :])
        return t

    def emit_seg(self, st: SegStage, src_dram, dst_dram, D, out_op):
        """One segment-sum stage. out_op in ('copy', 'relu')."""
        nc, mybir = self.nc, self.mybir
        f32, i16 = mybir.dt.float32, mybir.dt.int16
        idx_d = self.add_input(f"{st.name}_idx", [128, st.L // 16], i16,
                               st.idx_planes)
        tw_d = self.add_input(f"{st.name}_tw", [128, 2 * st.nblocks], f32,
                              st.tw_planes)

        sched = []  # block -> (window, j, is_last)
        for w in range(st.nw):
            for j in range(st.bpw[w]):
                sched.append((w, j, j == st.bpw[w] - 1))

        b = 0
        ps = None
        for start in range(0, st.nblocks, CH):
            nb = min(CH, st.nblocks - start)
            idx_t = self.p_meta.tile([128, nb * 8], i16, tag="idx")
            nc.sync.dma_start(idx_t[:],
                              idx_d[:, start * 8:(start + nb) * 8])
            tw_t = self.p_meta.tile([128, 2 * nb], f32, tag="tw")
            nc.sync.dma_start(tw_t[:],
                              tw_d[:, 2 * start:2 * (start + nb)])
            g_t = self.p_gath.tile([128, nb, D], f32, tag=f"g{D}")
            nc.gpsimd.dma_gather(
                g_t[:], src_dram[:, :], idx_t[:],
                num_idxs=nb * 128, num_idxs_reg=nb * 128, elem_size=D,
                single_packet=False)
            for k in range(nb):
                w, j, last = sched[b]
                if j == 0:
                    ps = self.p_ps.tile([128, D], f32, tag="seg")
                oh = self.p_oh.tile([128, 128], f32, tag="oh")
                nc.vector.tensor_scalar(
                    oh[:], self.iota_t[:],
                    tw_t[:, 2 * k:2 * k + 1], tw_t[:, 2 * k + 1:2 * k + 2],
                    mybir.AluOpType.is_equal, mybir.AluOpType.mult)
                nc.tensor.matmul(ps[:], oh[:], g_t[:, k, :],
                                 start=(j == 0), stop=last)
                if last:
                    r = self.p_fl.tile([128, D], f32, tag="fl")
                    if out_op == "relu":
                        nc.vector.tensor_scalar_max(r[:], ps[:], 0.0)
                    else:
                        nc.vector.tensor_copy(r[:], ps[:])
                    nc.sync.dma_start(dst_dram[128 * w:128 * (w + 1), :], r[:])
                b += 1

    def emit_linear(self, name, sources, Ws, bias_t, dst_dram, nchunks, Dout,
                    D=128):
        """dst chunk = sum_s source_s_chunk @ Ws[s] + bias.

        sources: list of (src_dram, None) for sequential 128-row chunks, or
        (src_dram, idx_dram) for rows gathered via a per-core index stream."""
        nc, mybir = self.nc, self.mybir
        f32, i16 = mybir.dt.float32, mybir.dt.int16
        GCH = 16  # chunks per gather group
        gtiles = {}

        def gathered_view(si, i, src_dram, idx_dram):
            grp = i // GCH
            if (si, grp) not in gtiles:
                n_in = min(GCH, nchunks - grp * GCH)
                idx_t = self.p_meta.tile([128, n_in * 8], i16, tag="lidx")
                nc.sync.dma_start(
                    idx_t[:],
                    idx_dram[:, grp * GCH * 8:(grp * GCH + n_in) * 8])
                g_t = self.p_gath.tile([128, n_in, D], f32, tag="lg")
                nc.gpsimd.dma_gather(
                    g_t[:], src_dram[:, :], idx_t[:],
                    num_idxs=n_in * 128, num_idxs_reg=n_in * 128, elem_size=D,
                    single_packet=False)
                gtiles[(si, grp)] = g_t
            return gtiles[(si, grp)][:, i % GCH, :]

        for i in range(nchunks):
            ps_lin = self.p_ps2.tile([128, Dout], f32, tag="lin")
            for si, (src, idx_dram) in enumerate(sources):
                if idx_dram is None:
                    ch = self.p_lin.tile([128, D], f32, tag="lch")
                    nc.sync.dma_start(ch[:], src[128 * i:128 * (i + 1), :])
                    src_view = ch[:]
                else:
                    src_view = gathered_view(si, i, src, idx_dram)
                ps_t = self.p_ps2.tile([128, D], f32, tag="tp")
                nc.tensor.transpose(ps_t[:], src_view, self.ident_t[:])
                tt = self.p_lin.tile([128, D], f32, tag="ltt")
                nc.vector.tensor_copy(tt[:], ps_t[:])
                nc.tensor.matmul(ps_lin[:], tt[:], Ws[si][:],
                                 start=(si == 0), stop=(si == len(sources) - 1))
            outt = self.p_lin.tile([128, Dout], f32, tag="lout")
            nc.vector.tensor_add(outt[:], ps_lin[:], bias_t[:])
            nc.sync.dma_start(dst_dram[128 * i:128 * (i + 1), :], outt[:])


# ---------------------------------------------------------------- main
def build(inputs, nphases=999):
    import concourse.bass as bass  # noqa: F401
    import concourse.tile as tile
    from concourse import bacc, mybir
    from contextlib import ExitStack

    X = np.ascontiguousarray(inputs["X"], np.float32)
    H = [
        (np.asarray(inputs["H0_v"]).astype(np.int64),
         np.asarray(inputs["H0_e"]).astype(np.int64), N0, E0),
        (np.asarray(inputs["H1_v"]).astype(np.int64),
         np.asarray(inputs["H1_e"]).astype(np.int64), N1, E1),
        (np.asarray(inputs["H2_v"]).astype(np.int64),
         np.asarray(inputs["H2_e"]).astype(np.int64), N2, E2),
    ]
    assign0 = np.asarray(inputs["assign0"]).astype(np.int64)
    assign1 = np.asarray(inputs["assign1"]).astype(np.int64)

    n0l, n1l, n2l = _pad_local(N0), _pad_local(N1), _pad_local(N2)
    e0p, e1p, e2p = _pad128(E0), _pad128(E1), _pad128(E2)

    def lap_streams(lv, nloc_pad):
        vi, ei, n, e = H[lv]
        dv_is, de_i = _degree_weights(vi, ei, n, e)
        owner, slot = vi % C, vi // C
        s1g, s1t, s1w, s2g, s2t, s2w = [], [], [], [], [], []
        for c in range(C):
            m = owner == c
            s1g.append(slot[m])
            s1t.append(ei[m])
            s1w.append(dv_is[vi[m]])
            s2g.append(ei[m])
            s2t.append(slot[m])
            s2w.append((dv_is[vi[m]] * de_i[ei[m]]).astype(np.float32))
        st1 = SegStage(f"l{lv}s1", s1g, s1t, s1w, _pad128(e))
        st2 = SegStage(f"l{lv}s2", s2g, s2t, s2w, nloc_pad)
        return st1, st2

    st1_0, st2_0 = lap_streams(0, n0l)
    st1_1, st2_1 = lap_streams(1, n1l)
    st1_2, st2_2 = lap_streams(2, n2l)

    def pool_streams(name, assign, nfine, ncoarse, ncl_pad):
        cnt = np.bincount(assign, minlength=ncoarse).astype(np.float32)
        inv = np.where(cnt > 0, 1.0 / cnt, 0.0).astype(np.float32)
        g = np.arange(nfine)
        owner, slot = g % C, g // C
        rows = (assign % C) * ncl_pad + assign // C
        gi, tr, wg = [], [], []
        for c in range(C):
            m = owner == c
            gi.append(slot[m])
            tr.append(rows[m])
            wg.append(inv[assign[m]])
        return SegStage(name, gi, tr, wg, C * ncl_pad)

    pool0 = pool_streams("pool0", assign0, N0, N1, n1l)
    pool1 = pool_streams("pool1", assign1, N1, N2, n2l)

    def unpool_planes(assign, nfine, ncl_pad):
        idxs = []
        for c in range(C):
            a = assign[np.arange(c, nfine, C)]
            idxs.append((a % C) * ncl_pad + a // C)
        return _gather_planes(idxs)

    up1_L, up1_planes = unpool_planes(assign1, N1, n2l)
    up0_L, up0_planes = unpool_planes(assign0, N0, n1l)

    nc = bacc.Bacc("TRN2", target_bir_lowering=False, debug=False,
                   num_devices=C)
    f32, i16 = mybir.dt.float32, mybir.dt.int16
    B = Builder(nc, mybir)

    x_arrs = []
    for c in range(C):
        xc = X[c::C]
        x_arrs.append(np.vstack([xc, np.zeros((n0l - len(xc), D_IN), np.float32)]))
    x_d = B.add_input("x", [n0l, D_IN], f32, x_arrs)
    out_d = nc.dram_tensor("out", [n0l, D_OUT], f32, kind="ExternalOutput")

    def dram(name, rows, d, shared=False):
        return nc.dram_tensor(name, [rows, d], f32,
                              addr_space="Shared" if shared else "Local")

    T0 = dram("T0", n0l, D_H)
    Y0p, Y0f = dram("Y0p", e0p, D_H), dram("Y0f", e0p, D_H, True)
    h0 = dram("h0", n0l, D_H)
    P1p, P1s = dram("P1p", C * n1l, D_H), dram("P1s", n1l, D_H)
    T1 = dram("T1", n1l, D_H)
    Y1p, Y1f = dram("Y1p", e1p, D_H), dram("Y1f", e1p, D_H, True)
    h1 = dram("h1", n1l, D_H)
    P2p, P2s = dram("P2p", C * n2l, D_H), dram("P2s", n2l, D_H)
    T2 = dram("T2", n2l, D_H)
    Y2p, Y2f = dram("Y2p", e2p, D_H), dram("Y2f", e2p, D_H, True)
    Xc2, Xc2f = dram("Xc2", n2l, D_H), dram("Xc2f", C * n2l, D_H, True)
    T3 = dram("T3", n1l, D_H)
    Y3p, Y3f = dram("Y3p", e1p, D_H), dram("Y3f", e1p, D_H, True)
    Xu1, Xuf = dram("Xu1", n1l, D_H), dram("Xuf", C * n1l, D_H, True)
    T4 = dram("T4", n0l, D_OUT)
    Y4p, Y4f = dram("Y4p", e0p, D_OUT), dram("Y4f", e0p, D_OUT, True)

    up1_d = B.add_input("up1_idx", [128, up1_L // 16], i16, up1_planes)
    up0_d = B.add_input("up0_idx", [128, up0_L // 16], i16, up0_planes)

    rg = [list(range(C))]

    def AR(src, dst):
        nc.gpsimd.collective_compute(
            "AllReduce", mybir.AluOpType.add, replica_groups=rg,
            ins=[src.ap().opt()], outs=[dst.ap().opt()])

    def RS(src, dst):
        nc.gpsimd.collective_compute(
            "ReduceScatter", mybir.AluOpType.add, replica_groups=rg,
            ins=[src.ap().opt()], outs=[dst.ap().opt()])

    def AG(src, dst):
        nc.gpsimd.collective_compute(
            "AllGather", mybir.AluOpType.bypass, replica_groups=rg,
            ins=[src.ap().opt()], outs=[dst.ap().opt()])

    with ExitStack() as ctx:
        tc = ctx.enter_context(tile.TileContext(nc))
        B.tc = tc
        B.setup_pools(ctx)
        B.setup_consts()
        W0t = B.const_mat("w0", inputs["W0"])
        W1t = B.const_mat("w1m", inputs["W1"])
        W2t = B.const_mat("w2m", inputs["W2"])
        W3a = B.const_mat("w3a", np.asarray(inputs["W3"])[:128])
        W3b = B.const_mat("w3b", np.asarray(inputs["W3"])[128:])
        W4a = B.const_mat("w4a", np.asarray(inputs["W4"])[:128])
        W4b = B.const_mat("w4b", np.asarray(inputs["W4"])[128:])
        b0t = B.const_mat("b0", np.tile(inputs["b0"], (128, 1)))
        b1t = B.const_mat("b1", np.tile(inputs["b1"], (128, 1)))
        b2t = B.const_mat("b2", np.tile(inputs["b2"], (128, 1)))
        b3t = B.const_mat("b3", np.tile(inputs["b3"], (128, 1)))
        b4t = B.const_mat("b4", np.tile(inputs["b4"], (128, 1)))

        phases = [
            lambda: B.emit_linear("lin0", [(x_d, None)], [W0t], b0t, T0, n0l // 128, D_H),
            lambda: B.emit_seg(st1_0, T0, Y0p, D_H, "copy"),
            lambda: AR(Y0p, Y0f),
            lambda: B.emit_seg(st2_0, Y0f, h0, D_H, "relu"),
            lambda: B.emit_seg(pool0, h0, P1p, D_H, "copy"),
            lambda: RS(P1p, P1s),
            lambda: B.emit_linear("lin1", [(P1s, None)], [W1t], b1t, T1, n1l // 128, D_H),
            lambda: B.emit_seg(st1_1, T1, Y1p, D_H, "copy"),
            lambda: AR(Y1p, Y1f),
            lambda: B.emit_seg(st2_1, Y1f, h1, D_H, "relu"),
            lambda: B.emit_seg(pool1, h1, P2p, D_H, "copy"),
            lambda: RS(P2p, P2s),
            lambda: B.emit_linear("lin2", [(P2s, None)], [W2t], b2t, T2, n2l // 128, D_H),
            lambda: B.emit_seg(st1_2, T2, Y2p, D_H, "copy"),
            lambda: AR(Y2p, Y2f),
            lambda: B.emit_seg(st2_2, Y2f, Xc2, D_H, "relu"),
            lambda: AG(Xc2, Xc2f),
            lambda: B.emit_linear("lin3", [(Xc2f, up1_d), (h1, None)], [W3a, W3b], b3t, T3, n1l // 128, D_H),
            lambda: B.emit_seg(st1_1.renamed("l1bs1"), T3, Y3p, D_H, "copy"),
            lambda: AR(Y3p, Y3f),
            lambda: B.emit_seg(st2_1.renamed("l1bs2"), Y3f, Xu1, D_H, "relu"),
            lambda: AG(Xu1, Xuf),
            lambda: B.emit_linear("lin4", [(Xuf, up0_d), (h0, None)], [W4a, W4b], b4t, T4, n0l // 128, D_OUT),
            lambda: B.emit_seg(st1_0.renamed("l0bs1"), T4, Y4p, D_OUT, "copy"),
            lambda: AR(Y4p, Y4f),
            lambda: B.emit_seg(st2_0.renamed("l0bs2"), Y4f, out_d, D_OUT, "copy"),
        ]
        for ph in phases[:nphases]:
            ph()
    nc.compile()

    in_maps = []
    for c in range(C):
        m = {}
        for name, arrs in B.inputs.items():
            m[name] = arrs[c] if isinstance(arrs, list) else arrs
        in_maps.append(m)
    return nc, in_maps


LAST_EXEC_NS = None


def _install_ntff_hook():
    import contextlib, ctypes, os, types
    try:
        from antenv import axon_hooks  # noqa: F401
        return
    except ImportError:
        pass
    import antenv
    so_path = os.environ.get("PJRT_LIBRARY_PATH", "/opt/axon/libaxon_pjrt.so")
    try:
        lib = ctypes.CDLL(so_path)
    except OSError:
        lib = None
    hook = None
    if lib is not None and hasattr(lib, "axon_start_nrt_profile"):
        lib.axon_start_nrt_profile.argtypes = [
            ctypes.POINTER(ctypes.c_int64), ctypes.c_size_t]
        lib.axon_start_nrt_profile.restype = ctypes.c_int64
        lib.axon_stop_nrt_profile.argtypes = [ctypes.c_char_p]
        lib.axon_stop_nrt_profile.restype = ctypes.c_int64

        @contextlib.contextmanager
        def hook(output_dir, device_ids):
            import jax
            jax.devices()
            if device_ids:
                ids = (ctypes.c_int64 * len(device_ids))(*device_ids)
                rc = lib.axon_start_nrt_profile(ids, len(device_ids))
            else:
                rc = lib.axon_start_nrt_profile(None, 0)
            if rc != 0:
                raise RuntimeError(f"axon_start_nrt_profile rc={rc}")
            try:
                yield
            finally:
                lib.axon_stop_nrt_profile(str(output_dir).encode())

    mod = types.ModuleType("antenv.axon_hooks")
    mod._hook = hook
    mod.get_axon_ntff_profile_hook = lambda: mod._hook
    def _set(h):
        mod._hook = h
    mod.set_axon_ntff_profile_hook = _set
    sys.modules["antenv.axon_hooks"] = mod
    antenv.axon_hooks = mod


def kernel(**inputs):
    global LAST_EXEC_NS
    import os
    trace = os.environ.get("HGNN_TRACE", "0") == "1"
    if trace:
        _install_ntff_hook()
    nc, in_maps = build(inputs)
    from concourse.bass_utils import run_bass_kernel_spmd
    res = run_bass_kernel_spmd(nc, in_maps, core_ids=list(range(C)),
                               trace=trace)
    LAST_EXEC_NS = res.exec_time_ns
    out = np.empty((N0, D_OUT), np.float32)
    for c in range(C):
        n = len(range(c, N0, C))
        out[c::C] = res.results[c]["out"][:n]
    return out



# revision 7
# speedup vs baseline: 1.0766x; 1.0766x over previous
"""Trainium2 Bass kernel for 3-level hierarchical hypergraph GNN (HGNN).

Strategy (8 NeuronCores, one SPMD NEFF, per-core index data):
  - Nodes of every level sharded round-robin: global id g -> core g%8, slot g//8.
  - Incidence entries assigned to the owner core of their node endpoint.
  - L_apply = two segment-sum passes:
      stage1 (edges): partial edge sums via dma_gather(node rows) + BINARY
                      one-hot matmul into 128-row PSUM edge windows; AllReduce.
      stage2 (nodes): gather full-edge-table rows + binary one-hot matmul into
                      local node windows (complete rows, no reduction needed).
    Degree scalings (Dv^-1/2, De^-1) are folded into per-row scales applied on
    the Activation engine (linear outputs x dv, stage1 outputs x de, stage2
    outputs x dv), so one-hots stay exactly {0,1}.
  - One-hots for a whole gather chunk are built with a single DVE
    tensor_tensor is_equal using stride-0 broadcast APs (iota vs target cols),
    avoiding the pathologically slow per-block tensor_scalar path.
  - bf16 tables + bf16 matmuls everywhere except the final D=64 L_apply
    (fp32; dma_gather requires elem >= 256B).
  - Pools/unpools: same machinery; ReduceScatter / AllGather for cluster maps.
"""
import sys

sys.path.insert(0, "/opt/trn_rl_repo")
import numpy as np
import ml_dtypes

BF16 = ml_dtypes.bfloat16

C = 8
CH = 64  # gather chunk size in 128-entry blocks (bf16 stages)
CHF = 32  # chunk size for fp32 stages

N0, N1, N2 = 100000, 25000, 6250
E0, E1, E2 = 20000, 5000, 1250
D_IN, D_H, D_OUT = 128, 128, 64


def _pad128(n):
    return ((n + 127) // 128) * 128


def _pad_local(n):
    return _pad128(-(-n // C))


# ---------------------------------------------------------------- host side
def _degree_weights(vi, ei, n, e):
    ones = np.ones(len(vi), np.float32)
    dV = np.bincount(vi, weights=ones, minlength=n)
    dE = np.bincount(ei, weights=ones, minlength=e)
    dv_is = np.where(dV > 0, dV ** -0.5, 0.0).astype(np.float32)
    de_i = np.where(dE > 0, 1.0 / dE, 0.0).astype(np.float32)
    return dv_is, de_i


def _plane_idx(idx):
    """int array (L,) -> [128, L//16] int16 (16-partition wrap, replicated x8)."""
    assert len(idx) % 16 == 0
    assert idx.max(initial=0) < 32768
    return np.tile(idx.astype(np.int16).reshape(-1, 16).T, (C, 1)).copy()


def _plane_tgt(tgt, valid):
    """-> [128, B] target col per (lane, block); -1 where padded."""
    nb = len(tgt) // 128
    t = np.where(valid, tgt.astype(np.float32), -1.0).astype(np.float32)
    return t.reshape(nb, 128).T.copy()


class SegStage:
    """Host data for one segment-sum stage, uniform structure across cores."""

    def __init__(self, name, gidx, trow, n_rows_padded):
        self.name = name
        self.nw = n_rows_padded // 128
        cnts = np.stack([
            np.bincount(trow[c] // 128, minlength=self.nw) for c in range(C)
        ])
        self.bpw = np.maximum(1, -(-cnts.max(axis=0) // 128)).astype(np.int64)
        self.nblocks = int(self.bpw.sum())
        L = 128 * self.nblocks
        self.L = L
        self.idx_planes, self.tgt_planes = [], []
        starts = np.concatenate([[0], np.cumsum(self.bpw[:-1])]) * 128
        for c in range(C):
            order = np.argsort(trow[c] // 128, kind="stable")
            gi = np.zeros(L, np.int64)
            tg = np.zeros(L, np.int64)
            va = np.zeros(L, bool)
            w_of = trow[c] // 128
            pos = np.searchsorted(w_of[order], np.arange(self.nw))
            end = np.searchsorted(w_of[order], np.arange(self.nw), side="right")
            for w in range(self.nw):
                sel = order[pos[w]:end[w]]
                s = starts[w]
                gi[s:s + len(sel)] = gidx[c][sel]
                tg[s:s + len(sel)] = trow[c][sel] % 128
                va[s:s + len(sel)] = True
            self.idx_planes.append(_plane_idx(gi))
            self.tgt_planes.append(_plane_tgt(tg, va))

    def renamed(self, name):
        st = SegStage.__new__(SegStage)
        st.__dict__ = dict(self.__dict__)
        st.name = name
        return st


def _gather_planes(idx_per_core):
    """Plain gather streams (no reduction), padded to a 128 multiple."""
    L = _pad128(len(idx_per_core[0]))
    planes = []
    for c in range(C):
        gi = np.zeros(L, np.int64)
        gi[: len(idx_per_core[c])] = idx_per_core[c]
        planes.append(_plane_idx(gi))
    return L, planes


# ---------------------------------------------------------------- device side
class Builder:
    def __init__(self, nc, mybir):
        self.nc = nc
        self.mybir = mybir
        self.tc = None
        self.inputs = {}  # name -> per-core list of arrays (or one shared array)

    def add_input(self, name, shape, dtype, arrays):
        assert name not in self.inputs, name
        t = self.nc.dram_tensor(name, list(shape), dtype, kind="ExternalInput")
        self.inputs[name] = arrays
        return t

    def setup_pools(self, ctx):
        tc = self.tc
        self.p_const = ctx.enter_context(tc.tile_pool(name="const", bufs=1))
        self.p_gath = ctx.enter_context(tc.tile_pool(name="gath", bufs=2))
        self.p_meta = ctx.enter_context(tc.tile_pool(name="meta", bufs=3))
        self.p_oh = ctx.enter_context(tc.tile_pool(name="oh", bufs=2))
        self.p_fl = ctx.enter_context(tc.tile_pool(name="fl", bufs=4))
        self.p_lin = ctx.enter_context(tc.tile_pool(name="lin", bufs=3))
        self.p_ps = ctx.enter_context(tc.tile_pool(name="ps", bufs=4, space="PSUM"))
        self.p_ps2 = ctx.enter_context(tc.tile_pool(name="ps2", bufs=2, space="PSUM"))

    def setup_consts(self):
        f32 = self.mybir.dt.float32
        bf16 = self.mybir.dt.bfloat16
        iota = np.tile(np.arange(128, dtype=np.float32), (128, 1))
        ident = np.eye(128, dtype=np.float32)
        self.iota_bf = self.const_mat("c_iotab", iota, bf16)
        self.iota_f = self.const_mat("c_iotaf", iota, f32)
        self.ident_bf = self.const_mat("c_identb", ident, bf16)

    def const_mat(self, name, arr, dt=None):
        mybir = self.mybir
        dt = dt if dt is not None else mybir.dt.float32
        if dt == mybir.dt.bfloat16:
            arr = np.ascontiguousarray(arr.astype(np.float32)).astype(BF16)
        else:
            arr = np.ascontiguousarray(arr, np.float32)
        d = self.add_input(name, list(arr.shape), dt, arr)
        t = self.p_const.tile(list(arr.shape), dt, tag=name)
        self.nc.sync.dma_start(t[:], d[:, :])
        return t

    def emit_seg(self, st: SegStage, src_dram, dst_dram, D, out_op, scale_t,
                 fp32=False):
        """One segment-sum stage. out_op in ('copy', 'relu').

        scale_t: const tile [128, st.nw]; output window w is scaled per-row by
        scale_t[:, w] on the Activation engine.
        """
        nc, mybir = self.nc, self.mybir
        f32, i16 = mybir.dt.float32, mybir.dt.int16
        bf16 = mybir.dt.bfloat16
        dt = f32 if fp32 else bf16
        iota_t = self.iota_f if fp32 else self.iota_bf
        ch = CHF if fp32 else CH
        idx_d = self.add_input(f"{st.name}_idx", [128, st.L // 16], i16,
                               st.idx_planes)
        tgt_d = self.add_input(
            f"{st.name}_tg", [128, st.nblocks], dt,
            [p if fp32 else p.astype(BF16) for p in st.tgt_planes])
        func_relu = mybir.ActivationFunctionType.Relu
        func_copy = mybir.ActivationFunctionType.Copy

        sched = []  # block -> (window, j, is_last)
        for w in range(st.nw):
            for j in range(st.bpw[w]):
                sched.append((w, j, j == st.bpw[w] - 1))

        b = 0
        ps = None
        for start in range(0, st.nblocks, ch):
            nb = min(ch, st.nblocks - start)
            idx_t = self.p_meta.tile([128, nb * 8], i16, tag="idx")
            nc.sync.dma_start(idx_t[:],
                              idx_d[:, start * 8:(start + nb) * 8])
            tgt_t = self.p_meta.tile([128, nb], dt, tag="tg")
            nc.sync.dma_start(tgt_t[:], tgt_d[:, start:start + nb])
            g_t = self.p_gath.tile([128, nb, D], dt, tag=f"g{D}{dt}")
            nc.gpsimd.dma_gather(
                g_t[:], src_dram[:, :], idx_t[:],
                num_idxs=nb * 128, num_idxs_reg=nb * 128, elem_size=D,
                single_packet=False)
            oh_t = self.p_oh.tile([128, nb, 128], dt, tag=f"oh{dt}")
            nc.vector.tensor_tensor(
                oh_t[:],
                iota_t[:].unsqueeze(1).to_broadcast([128, nb, 128]),
                tgt_t[:].unsqueeze(2).to_broadcast([128, nb, 128]),
                mybir.AluOpType.is_equal)
            for k in range(nb):
                w, j, last = sched[b]
                if j == 0:
                    ps = self.p_ps.tile([128, D], f32, tag="seg")
                nc.tensor.matmul(ps[:], oh_t[:, k, :], g_t[:, k, :],
                                 start=(j == 0), stop=last)
                if last:
                    r = self.p_fl.tile([128, D], dst_dram.dtype, tag="fl")
                    nc.scalar.activation(
                        r[:], ps[:],
                        func_relu if out_op == "relu" else func_copy,
                        scale=scale_t[:, w:w + 1])
                    nc.sync.dma_start(dst_dram[128 * w:128 * (w + 1), :], r[:])
                b += 1

    def emit_linear(self, name, sources, Ws, bias, dst_dram, nchunks, Dout,
                    scale_t=None, D=128):
        """dst chunk = (sum_s source_s_chunk @ Ws[s] + bias) * scale_row.

        sources: list of (src_dram, None) for sequential 128-row chunks, or
        (src_dram, idx_dram) for rows gathered via a per-core index stream.
        bias: np vector or None. scale_t: const tile [128, nchunks] or None."""
        nc, mybir = self.nc, self.mybir
        f32, i16 = mybir.dt.float32, mybir.dt.int16
        bf16 = mybir.dt.bfloat16
        GCH = 16  # chunks per gather group
        gtiles = {}
        bias_t = None
        if bias is not None and np.any(np.asarray(bias) != 0):
            bias_t = self.const_mat(f"{name}_b", np.tile(bias, (128, 1)))
        func_copy = mybir.ActivationFunctionType.Copy

        def gathered_view(si, i, src_dram, idx_dram):
            grp = i // GCH
            if (si, grp) not in gtiles:
                n_in = min(GCH, nchunks - grp * GCH)
                idx_t = self.p_meta.tile([128, n_in * 8], i16, tag="lidx")
                nc.sync.dma_start(
                    idx_t[:],
                    idx_dram[:, grp * GCH * 8:(grp * GCH + n_in) * 8])
                g_t = self.p_gath.tile([128, n_in, D], bf16, tag="lg")
                nc.gpsimd.dma_gather(
                    g_t[:], src_dram[:, :], idx_t[:],
                    num_idxs=n_in * 128, num_idxs_reg=n_in * 128, elem_size=D,
                    single_packet=False)
                gtiles[(si, grp)] = g_t
            return gtiles[(si, grp)][:, i % GCH, :]

        for i in range(nchunks):
            ps_lin = self.p_ps2.tile([128, Dout], f32, tag="lin")
            for si, (src, idx_dram) in enumerate(sources):
                if idx_dram is None:
                    chk = self.p_lin.tile([128, D], bf16, tag="lch")
                    nc.sync.dma_start(chk[:], src[128 * i:128 * (i + 1), :])
                    src_view = chk[:]
                else:
                    src_view = gathered_view(si, i, src, idx_dram)
                ps_t = self.p_ps2.tile([128, D], bf16, tag="tp")
                nc.tensor.transpose(ps_t[:], src_view, self.ident_bf[:])
                tt = self.p_lin.tile([128, D], bf16, tag="ltt")
                nc.vector.tensor_copy(tt[:], ps_t[:])
                nc.tensor.matmul(ps_lin[:], tt[:], Ws[si][:],
                                 start=(si == 0), stop=(si == len(sources) - 1))
            outt = self.p_lin.tile([128, Dout], dst_dram.dtype, tag="lout")
            if bias_t is not None:
                bsum = self.p_lin.tile([128, Dout], f32, tag="lbs")
                nc.vector.tensor_add(bsum[:], ps_lin[:], bias_t[:])
                src_ap = bsum[:]
            else:
                src_ap = ps_lin[:]
            nc.scalar.activation(
                outt[:], src_ap, func_copy,
                scale=(scale_t[:, i:i + 1] if scale_t is not None else 1.0))
            nc.sync.dma_start(dst_dram[128 * i:128 * (i + 1), :], outt[:])


# ---------------------------------------------------------------- main
def build(inputs, nphases=999, do_compile=True):
    import concourse.bass as bass  # noqa: F401
    import concourse.tile as tile
    from concourse import bacc, mybir
    from contextlib import ExitStack

    X = np.ascontiguousarray(inputs["X"], np.float32)
    H = [
        (np.asarray(inputs["H0_v"]).astype(np.int64),
         np.asarray(inputs["H0_e"]).astype(np.int64), N0, E0),
        (np.asarray(inputs["H1_v"]).astype(np.int64),
         np.asarray(inputs["H1_e"]).astype(np.int64), N1, E1),
        (np.asarray(inputs["H2_v"]).astype(np.int64),
         np.asarray(inputs["H2_e"]).astype(np.int64), N2, E2),
    ]
    assign0 = np.asarray(inputs["assign0"]).astype(np.int64)
    assign1 = np.asarray(inputs["assign1"]).astype(np.int64)

    n0l, n1l, n2l = _pad_local(N0), _pad_local(N1), _pad_local(N2)
    e0p, e1p, e2p = _pad128(E0), _pad128(E1), _pad128(E2)

    dv_planes, de_planes = [], []

    def lap_streams(lv, nloc_pad):
        vi, ei, n, e = H[lv]
        dv_is, de_i = _degree_weights(vi, ei, n, e)
        owner, slot = vi % C, vi // C
        s1g, s1t, s2g, s2t = [], [], [], []
        for c in range(C):
            m = owner == c
            s1g.append(slot[m])
            s1t.append(ei[m])
            s2g.append(ei[m])
            s2t.append(slot[m])
        st1 = SegStage(f"l{lv}s1", s1g, s1t, _pad128(e))
        st2 = SegStage(f"l{lv}s2", s2g, s2t, nloc_pad)
        # per-core dv plane over local slots; de plane over edge windows
        dvp = []
        for c in range(C):
            loc = np.zeros(nloc_pad, np.float32)
            ids = np.arange(c, n, C)
            loc[: len(ids)] = dv_is[ids]
            dvp.append(loc.reshape(-1, 128).T.copy())
        dep = np.zeros(_pad128(e), np.float32)
        dep[:e] = de_i
        dep = dep.reshape(-1, 128).T.copy()
        dv_planes.append(dvp)
        de_planes.append(dep)
        return st1, st2

    st1_0, st2_0 = lap_streams(0, n0l)
    st1_1, st2_1 = lap_streams(1, n1l)
    st1_2, st2_2 = lap_streams(2, n2l)

    def pool_streams(name, assign, nfine, ncoarse, ncl_pad):
        cnt = np.bincount(assign, minlength=ncoarse).astype(np.float32)
        inv = np.where(cnt > 0, 1.0 / cnt, 0.0).astype(np.float32)
        g = np.arange(nfine)
        owner, slot = g % C, g // C
        rows = (assign % C) * ncl_pad + assign // C
        gi, tr = [], []
        for c in range(C):
            m = owner == c
            gi.append(slot[m])
            tr.append(rows[m])
        st = SegStage(name, gi, tr, C * ncl_pad)
        # inverse-count scale plane over C*ncl_pad target rows (shared)
        full = np.zeros(C * ncl_pad, np.float32)
        rows_all = (np.arange(ncoarse) % C) * ncl_pad + np.arange(ncoarse) // C
        full[rows_all] = inv
        icp = full.reshape(-1, 128).T.copy()
        return st, icp

    pool0, ic1p = pool_streams("pool0", assign0, N0, N1, n1l)
    pool1, ic2p = pool_streams("pool1", assign1, N1, N2, n2l)

    def unpool_planes(assign, nfine, ncl_pad):
        idxs = []
        for c in range(C):
            a = assign[np.arange(c, nfine, C)]
            idxs.append((a % C) * ncl_pad + a // C)
        return _gather_planes(idxs)

    up1_L, up1_planes = unpool_planes(assign1, N1, n2l)
    up0_L, up0_planes = unpool_planes(assign0, N0, n1l)

    nc = bacc.Bacc("TRN2", target_bir_lowering=False, debug=False,
                   num_devices=C)
    f32, i16 = mybir.dt.float32, mybir.dt.int16
    bf16 = mybir.dt.bfloat16
    B = Builder(nc, mybir)

    x_arrs = []
    for c in range(C):
        xc = X[c::C]
        xc = np.vstack([xc, np.zeros((n0l - len(xc), D_IN), np.float32)])
        x_arrs.append(xc.astype(BF16))
    x_d = B.add_input("x", [n0l, D_IN], bf16, x_arrs)
    out_d = nc.dram_tensor("out", [n0l, D_OUT], f32, kind="ExternalOutput")

    def dram(name, rows, d, dt=bf16, shared=False):
        return nc.dram_tensor(name, [rows, d], dt,
                              addr_space="Shared" if shared else "Local")

    T0 = dram("T0", n0l, D_H)
    Y0p, Y0f = dram("Y0p", e0p, D_H), dram("Y0f", e0p, D_H, shared=True)
    h0 = dram("h0", n0l, D_H)
    P1p, P1s = dram("P1p", C * n1l, D_H), dram("P1s", n1l, D_H)
    T1 = dram("T1", n1l, D_H)
    Y1p, Y1f = dram("Y1p", e1p, D_H), dram("Y1f", e1p, D_H, shared=True)
    h1 = dram("h1", n1l, D_H)
    P2p, P2s = dram("P2p", C * n2l, D_H), dram("P2s", n2l, D_H)
    T2 = dram("T2", n2l, D_H)
    Y2p, Y2f = dram("Y2p", e2p, D_H), dram("Y2f", e2p, D_H, shared=True)
    Xc2, Xc2f = dram("Xc2", n2l, D_H), dram("Xc2f", C * n2l, D_H, shared=True)
    T3 = dram("T3", n1l, D_H)
    Y3p, Y3f = dram("Y3p", e1p, D_H), dram("Y3f", e1p, D_H, shared=True)
    Xu1, Xuf = dram("Xu1", n1l, D_H), dram("Xuf", C * n1l, D_H, shared=True)
    T4 = dram("T4", n0l, D_OUT, dt=f32)
    Y4p = dram("Y4p", e0p, D_OUT, dt=f32)
    Y4f = dram("Y4f", e0p, D_OUT, dt=f32, shared=True)

    up1_d = B.add_input("up1_idx", [128, up1_L // 16], i16, up1_planes)
    up0_d = B.add_input("up0_idx", [128, up0_L // 16], i16, up0_planes)

    rg = [list(range(C))]

    def AR(src, dst):
        nc.gpsimd.collective_compute(
            "AllReduce", mybir.AluOpType.add, replica_groups=rg,
            ins=[src.ap().opt()], outs=[dst.ap().opt()])

    def RS(src, dst):
        nc.gpsimd.collective_compute(
            "ReduceScatter", mybir.AluOpType.add, replica_groups=rg,
            ins=[src.ap().opt()], outs=[dst.ap().opt()])

    def AG(src, dst):
        nc.gpsimd.collective_compute(
            "AllGather", mybir.AluOpType.bypass, replica_groups=rg,
            ins=[src.ap().opt()], outs=[dst.ap().opt()])

    with ExitStack() as ctx:
        tc = ctx.enter_context(tile.TileContext(nc))
        B.tc = tc
        B.setup_pools(ctx)
        B.setup_consts()
        W0t = B.const_mat("w0", np.asarray(inputs["W0"]), bf16)
        W1t = B.const_mat("w1m", np.asarray(inputs["W1"]), bf16)
        W2t = B.const_mat("w2m", np.asarray(inputs["W2"]), bf16)
        W3a = B.const_mat("w3a", np.asarray(inputs["W3"])[:128], bf16)
        W3b = B.const_mat("w3b", np.asarray(inputs["W3"])[128:], bf16)
        W4a = B.const_mat("w4a", np.asarray(inputs["W4"])[:128], bf16)
        W4b = B.const_mat("w4b", np.asarray(inputs["W4"])[128:], bf16)
        dv0 = B.add_input("dv0p", [128, n0l // 128], f32, dv_planes[0])
        dv1 = B.add_input("dv1p", [128, n1l // 128], f32, dv_planes[1])
        dv2 = B.add_input("dv2p", [128, n2l // 128], f32, dv_planes[2])
        de0 = B.add_input("de0p", [128, e0p // 128], f32, de_planes[0])
        de1 = B.add_input("de1p", [128, e1p // 128], f32, de_planes[1])
        de2 = B.add_input("de2p", [128, e2p // 128], f32, de_planes[2])
        ic1 = B.add_input("ic1p", [128, C * n1l // 128], f32, ic1p)
        ic2 = B.add_input("ic2p", [128, C * n2l // 128], f32, ic2p)

        def load_plane(d, cols, tag):
            t = B.p_const.tile([128, cols], f32, tag=tag)
            nc.sync.dma_start(t[:], d[:, :])
            return t

        dv0t = load_plane(dv0, n0l // 128, "dv0t")
        dv1t = load_plane(dv1, n1l // 128, "dv1t")
        dv2t = load_plane(dv2, n2l // 128, "dv2t")
        de0t = load_plane(de0, e0p // 128, "de0t")
        de1t = load_plane(de1, e1p // 128, "de1t")
        de2t = load_plane(de2, e2p // 128, "de2t")
        ic1t = load_plane(ic1, C * n1l // 128, "ic1t")
        ic2t = load_plane(ic2, C * n2l // 128, "ic2t")

        ins = inputs
        phases = [
            lambda: B.emit_linear("lin0", [(x_d, None)], [W0t], ins["b0"], T0,
                                  n0l // 128, D_H, scale_t=dv0t),
            lambda: B.emit_seg(st1_0, T0, Y0p, D_H, "copy", de0t),
            lambda: AR(Y0p, Y0f),
            lambda: B.emit_seg(st2_0, Y0f, h0, D_H, "relu", dv0t),
            lambda: B.emit_seg(pool0, h0, P1p, D_H, "copy", ic1t),
            lambda: RS(P1p, P1s),
            lambda: B.emit_linear("lin1", [(P1s, None)], [W1t], ins["b1"], T1,
                                  n1l // 128, D_H, scale_t=dv1t),
            lambda: B.emit_seg(st1_1, T1, Y1p, D_H, "copy", de1t),
            lambda: AR(Y1p, Y1f),
            lambda: B.emit_seg(st2_1, Y1f, h1, D_H, "relu", dv1t),
            lambda: B.emit_seg(pool1, h1, P2p, D_H, "copy", ic2t),
            lambda: RS(P2p, P2s),
            lambda: B.emit_linear("lin2", [(P2s, None)], [W2t], ins["b2"], T2,
                                  n2l // 128, D_H, scale_t=dv2t),
            lambda: B.emit_seg(st1_2, T2, Y2p, D_H, "copy", de2t),
            lambda: AR(Y2p, Y2f),
            lambda: B.emit_seg(st2_2, Y2f, Xc2, D_H, "relu", dv2t),
            lambda: AG(Xc2, Xc2f),
            lambda: B.emit_linear("lin3", [(Xc2f, up1_d), (h1, None)],
                                  [W3a, W3b], ins["b3"], T3, n1l // 128, D_H,
                                  scale_t=dv1t),
            lambda: B.emit_seg(st1_1.renamed("l1bs1"), T3, Y3p, D_H, "copy",
                               de1t),
            lambda: AR(Y3p, Y3f),
            lambda: B.emit_seg(st2_1.renamed("l1bs2"), Y3f, Xu1, D_H, "relu",
                               dv1t),
            lambda: AG(Xu1, Xuf),
            lambda: B.emit_linear("lin4", [(Xuf, up0_d), (h0, None)],
                                  [W4a, W4b], ins["b4"], T4, n0l // 128, D_OUT,
                                  scale_t=dv0t),
            lambda: B.emit_seg(st1_0.renamed("l0bs1"), T4, Y4p, D_OUT, "copy",
                               de0t, fp32=True),
            lambda: AR(Y4p, Y4f),
            lambda: B.emit_seg(st2_0.renamed("l0bs2"), Y4f, out_d, D_OUT,
                               "copy", dv0t, fp32=True),
        ]
        for ph in phases[:nphases]:
            ph()
    if do_compile:
        nc.compile()

    in_maps = []
    for c in range(C):
        m = {}
        for name, arrs in B.inputs.items():
            m[name] = arrs[c] if isinstance(arrs, list) else arrs
        in_maps.append(m)
    return nc, in_maps


LAST_EXEC_NS = None


def _install_ntff_hook():
    import contextlib, ctypes, os, types
    try:
        from antenv import axon_hooks  # noqa: F401
        return
    except ImportError:
        pass
    import antenv
    so_path = os.environ.get("PJRT_LIBRARY_PATH", "/opt/axon/libaxon_pjrt.so")
    try:
        lib = ctypes.CDLL(so_path)
    except OSError:
        lib = None
    hook = None
    if lib is not None and hasattr(lib, "axon_start_nrt_profile"):
        lib.axon_start_nrt_profile.argtypes = [
            ctypes.POINTER(ctypes.c_int64), ctypes.c_size_t]
        lib.axon_start_nrt_profile.restype = ctypes.c_int64
        lib.axon_stop_nrt_profile.argtypes = [ctypes.c_char_p]
        lib.axon_stop_nrt_profile.restype = ctypes.c_int64

        @contextlib.contextmanager
        def hook(output_dir, device_ids):
            import jax
            jax.devices()
            if device_ids:
                ids = (ctypes.c_int64 * len(device_ids))(*device_ids)
                rc = lib.axon_start_nrt_profile(ids, len(device_ids))
            else:
                rc = lib.axon_start_nrt_profile(None, 0)
            if rc != 0:
                raise RuntimeError(f"axon_start_nrt_profile rc={rc}")
            try:
                yield
            finally:
                lib.axon_stop_nrt_profile(str(output_dir).encode())

    mod = types.ModuleType("antenv.axon_hooks")
    mod._hook = hook
    mod.get_axon_ntff_profile_hook = lambda: mod._hook
    def _set(h):
        mod._hook = h
    mod.set_axon_ntff_profile_hook = _set
    sys.modules["antenv.axon_hooks"] = mod
    antenv.axon_hooks = mod


def kernel(**inputs):
    global LAST_EXEC_NS
    import os
    trace = os.environ.get("HGNN_TRACE", "0") == "1"
    if trace:
        _install_ntff_hook()
    nc, in_maps = build(inputs)
    from concourse.bass_utils import run_bass_kernel_spmd
    res = run_bass_kernel_spmd(nc, in_maps, core_ids=list(range(C)),
                               trace=trace)
    LAST_EXEC_NS = res.exec_time_ns
    out = np.empty((N0, D_OUT), np.float32)
    for c in range(C):
        n = len(range(c, N0, C))
        out[c::C] = res.results[c]["out"][:n]
    return out


# revision 11
# speedup vs baseline: 1.0831x; 1.0060x over previous
"""Trainium2 Bass kernel for 3-level hierarchical hypergraph GNN (HGNN).

Strategy (8 NeuronCores, one SPMD NEFF, per-core index data):
  - Nodes of every level sharded round-robin: global id g -> core g%8, slot g//8.
  - Incidence entries assigned to the owner core of their node endpoint.
  - L_apply = two segment-sum passes:
      stage1 (edges): partial edge sums via dma_gather(node rows) + BINARY
                      one-hot matmul into 128-row PSUM edge windows; AllReduce.
      stage2 (nodes): gather full-edge-table rows + binary one-hot matmul into
                      local node windows (complete rows, no reduction needed).
    Degree scalings (Dv^-1/2, De^-1) are folded into per-row scales applied on
    the Activation engine (linear outputs x dv, stage1 outputs x de, stage2
    outputs x dv), so one-hots stay exactly {0,1}.
  - One-hots for a whole gather chunk are built with a single DVE
    tensor_tensor is_equal using stride-0 broadcast APs (iota vs target cols),
    avoiding the pathologically slow per-block tensor_scalar path.
  - bf16 tables + bf16 matmuls everywhere except the final D=64 L_apply
    (fp32; dma_gather requires elem >= 256B).
  - Pools/unpools: same machinery; ReduceScatter / AllGather for cluster maps.
"""
import sys

sys.path.insert(0, "/opt/trn_rl_repo")
import numpy as np
import ml_dtypes

BF16 = ml_dtypes.bfloat16

C = 8
CH = 64  # gather chunk size in 128-entry blocks (bf16 stages)
CHF = 32  # chunk size for fp32 stages

N0, N1, N2 = 100000, 25000, 6250
E0, E1, E2 = 20000, 5000, 1250
D_IN, D_H, D_OUT = 128, 128, 64


def _pad128(n):
    return ((n + 127) // 128) * 128


def _pad_local(n):
    return _pad128(-(-n // C))


# ---------------------------------------------------------------- host side
def _balance_edges(vi, ei, e):
    """Renumber edges so per-(core,window) stage-1 entry counts are balanced.

    Returns perm with new_id = perm[old_id]."""
    ep = _pad128(e)
    nw = ep // 128
    d = np.zeros((C, e), np.int64)
    for c in range(C):
        m = (vi % C) == c
        d[c] = np.bincount(ei[m], minlength=e)
    order = np.argsort(-d.sum(axis=0), kind="stable")
    load = np.zeros((C, nw), np.int64)
    cap = np.full(nw, 128, np.int64)
    cap[nw - 1] = 128 - (ep - e)
    perm = np.empty(e, np.int64)
    slot_next = np.zeros(nw, np.int64)
    for eid in order:
        cand = cap > 0
        score = (load[:, cand] + d[:, eid][:, None]).max(axis=0)
        w = np.nonzero(cand)[0][np.argmin(score)]
        load[:, w] += d[:, eid]
        perm[eid] = w * 128 + slot_next[w]
        slot_next[w] += 1
        cap[w] -= 1
    return perm


def _degree_weights(vi, ei, n, e):
    ones = np.ones(len(vi), np.float32)
    dV = np.bincount(vi, weights=ones, minlength=n)
    dE = np.bincount(ei, weights=ones, minlength=e)
    dv_is = np.where(dV > 0, dV ** -0.5, 0.0).astype(np.float32)
    de_i = np.where(dE > 0, 1.0 / dE, 0.0).astype(np.float32)
    return dv_is, de_i


def _plane_idx(idx):
    """int array (L,) -> [128, L//16] int16 (16-partition wrap, replicated x8)."""
    assert len(idx) % 16 == 0
    assert idx.max(initial=0) < 32768
    return np.tile(idx.astype(np.int16).reshape(-1, 16).T, (C, 1)).copy()


def _plane_tgt(tgt, valid):
    """-> [128, B] target col per (lane, block); -1 where padded."""
    nb = len(tgt) // 128
    t = np.where(valid, tgt.astype(np.float32), -1.0).astype(np.float32)
    return t.reshape(nb, 128).T.copy()


class SegStage:
    """Host data for one segment-sum stage, uniform structure across cores."""

    def __init__(self, name, gidx, trow, n_rows_padded):
        self.name = name
        self.nw = n_rows_padded // 128
        cnts = np.stack([
            np.bincount(trow[c] // 128, minlength=self.nw) for c in range(C)
        ])
        self.bpw = np.maximum(1, -(-cnts.max(axis=0) // 128)).astype(np.int64)
        self.nblocks = int(self.bpw.sum())
        L = 128 * self.nblocks
        self.L = L
        self.idx_planes, self.tgt_planes = [], []
        starts = np.concatenate([[0], np.cumsum(self.bpw[:-1])]) * 128
        for c in range(C):
            order = np.argsort(trow[c] // 128, kind="stable")
            gi = np.zeros(L, np.int64)
            tg = np.zeros(L, np.int64)
            va = np.zeros(L, bool)
            w_of = trow[c] // 128
            pos = np.searchsorted(w_of[order], np.arange(self.nw))
            end = np.searchsorted(w_of[order], np.arange(self.nw), side="right")
            for w in range(self.nw):
                sel = order[pos[w]:end[w]]
                s = starts[w]
                gi[s:s + len(sel)] = gidx[c][sel]
                tg[s:s + len(sel)] = trow[c][sel] % 128
                va[s:s + len(sel)] = True
            self.idx_planes.append(_plane_idx(gi))
            self.tgt_planes.append(_plane_tgt(tg, va))

    def renamed(self, name):
        st = SegStage.__new__(SegStage)
        st.__dict__ = dict(self.__dict__)
        st.name = name
        return st


def _gather_planes(idx_per_core):
    """Plain gather streams (no reduction), padded to a 128 multiple."""
    L = _pad128(len(idx_per_core[0]))
    planes = []
    for c in range(C):
        gi = np.zeros(L, np.int64)
        gi[: len(idx_per_core[c])] = idx_per_core[c]
        planes.append(_plane_idx(gi))
    return L, planes


# ---------------------------------------------------------------- device side
class Builder:
    def __init__(self, nc, mybir):
        self.nc = nc
        self.mybir = mybir
        self.tc = None
        self.inputs = {}  # name -> per-core list of arrays (or one shared array)

    def add_input(self, name, shape, dtype, arrays):
        assert name not in self.inputs, name
        t = self.nc.dram_tensor(name, list(shape), dtype, kind="ExternalInput")
        self.inputs[name] = arrays
        return t

    def setup_pools(self, ctx):
        tc = self.tc
        self.p_const = ctx.enter_context(tc.tile_pool(name="const", bufs=1))
        self.p_gath = ctx.enter_context(tc.tile_pool(name="gath", bufs=2))
        self.p_meta = ctx.enter_context(tc.tile_pool(name="meta", bufs=3))
        self.p_oh = ctx.enter_context(tc.tile_pool(name="oh", bufs=2))
        self.p_fl = ctx.enter_context(tc.tile_pool(name="fl", bufs=4))
        self.p_lin = ctx.enter_context(tc.tile_pool(name="lin", bufs=3))
        self.p_ps = ctx.enter_context(tc.tile_pool(name="ps", bufs=4, space="PSUM"))
        self.p_ps2 = ctx.enter_context(tc.tile_pool(name="ps2", bufs=2, space="PSUM"))

    def setup_consts(self):
        f32 = self.mybir.dt.float32
        bf16 = self.mybir.dt.bfloat16
        iota = np.tile(np.arange(128, dtype=np.float32), (128, 1))
        ident = np.eye(128, dtype=np.float32)
        self.iota_bf = self.const_mat("c_iotab", iota, bf16)
        self.iota_f = self.const_mat("c_iotaf", iota, f32)
        self.ident_bf = self.const_mat("c_identb", ident, bf16)

    def const_mat(self, name, arr, dt=None):
        mybir = self.mybir
        dt = dt if dt is not None else mybir.dt.float32
        if dt == mybir.dt.bfloat16:
            arr = np.ascontiguousarray(arr.astype(np.float32)).astype(BF16)
        else:
            arr = np.ascontiguousarray(arr, np.float32)
        d = self.add_input(name, list(arr.shape), dt, arr)
        t = self.p_const.tile(list(arr.shape), dt, tag=name)
        self.nc.sync.dma_start(t[:], d[:, :])
        return t

    def emit_seg(self, st: SegStage, src_dram, dst_dram, D, out_op, scale_t,
                 fp32=False):
        """One segment-sum stage. out_op in ('copy', 'relu').

        scale_t: const tile [128, st.nw]; output window w is scaled per-row by
        scale_t[:, w] on the Activation engine.
        """
        nc, mybir = self.nc, self.mybir
        f32, i16 = mybir.dt.float32, mybir.dt.int16
        bf16 = mybir.dt.bfloat16
        dt = f32 if fp32 else bf16
        iota_t = self.iota_f if fp32 else self.iota_bf
        ch = CHF if fp32 else CH
        idx_d = self.add_input(f"{st.name}_idx", [128, st.L // 16], i16,
                               st.idx_planes)
        tgt_d = self.add_input(
            f"{st.name}_tg", [128, st.nblocks], dt,
            [p if fp32 else p.astype(BF16) for p in st.tgt_planes])
        func_relu = mybir.ActivationFunctionType.Relu
        func_copy = mybir.ActivationFunctionType.Copy

        sched = []  # block -> (window, j, is_last)
        for w in range(st.nw):
            for j in range(st.bpw[w]):
                sched.append((w, j, j == st.bpw[w] - 1))

        b = 0
        ps = None
        for start in range(0, st.nblocks, ch):
            nb = min(ch, st.nblocks - start)
            idx_t = self.p_meta.tile([128, nb * 8], i16, tag="idx")
            nc.sync.dma_start(idx_t[:],
                              idx_d[:, start * 8:(start + nb) * 8])
            tgt_t = self.p_meta.tile([128, nb], dt, tag="tg")
            nc.sync.dma_start(tgt_t[:], tgt_d[:, start:start + nb])
            g_t = self.p_gath.tile([128, nb, D], dt, tag=f"g{D}{dt}")
            nc.gpsimd.dma_gather(
                g_t[:], src_dram[:, :], idx_t[:],
                num_idxs=nb * 128, num_idxs_reg=nb * 128, elem_size=D,
                single_packet=False)
            oh_t = self.p_oh.tile([128, nb, 128], dt, tag=f"oh{dt}")
            nc.vector.tensor_tensor(
                oh_t[:],
                iota_t[:].unsqueeze(1).to_broadcast([128, nb, 128]),
                tgt_t[:].unsqueeze(2).to_broadcast([128, nb, 128]),
                mybir.AluOpType.is_equal)
            for k in range(nb):
                w, j, last = sched[b]
                if j == 0:
                    ps = self.p_ps.tile([128, D], f32, tag="seg")
                nc.tensor.matmul(ps[:], oh_t[:, k, :], g_t[:, k, :],
                                 start=(j == 0), stop=last)
                if last:
                    r = self.p_fl.tile([128, D], dst_dram.dtype, tag="fl")
                    nc.scalar.activation(
                        r[:], ps[:],
                        func_relu if out_op == "relu" else func_copy,
                        scale=scale_t[:, w:w + 1])
                    nc.sync.dma_start(dst_dram[128 * w:128 * (w + 1), :], r[:])
                b += 1

    def emit_linear(self, name, sources, Ws, bias, dst_dram, nchunks, Dout,
                    scale_t=None, D=128):
        """dst chunk = (sum_s source_s_chunk @ Ws[s] + bias) * scale_row.

        sources: list of (src_dram, None) for sequential 128-row chunks, or
        (src_dram, idx_dram) for rows gathered via a per-core index stream.
        bias: np vector or None. scale_t: const tile [128, nchunks] or None."""
        nc, mybir = self.nc, self.mybir
        f32, i16 = mybir.dt.float32, mybir.dt.int16
        bf16 = mybir.dt.bfloat16
        GCH = 16  # chunks per gather group
        gtiles = {}
        bias_t = None
        if bias is not None and np.any(np.asarray(bias) != 0):
            bias_t = self.const_mat(f"{name}_b", np.tile(bias, (128, 1)))
        func_copy = mybir.ActivationFunctionType.Copy

        def gathered_view(si, i, src_dram, idx_dram):
            grp = i // GCH
            if (si, grp) not in gtiles:
                n_in = min(GCH, nchunks - grp * GCH)
                idx_t = self.p_meta.tile([128, n_in * 8], i16, tag="lidx")
                nc.sync.dma_start(
                    idx_t[:],
                    idx_dram[:, grp * GCH * 8:(grp * GCH + n_in) * 8])
                g_t = self.p_gath.tile([128, n_in, D], bf16, tag="lg")
                nc.gpsimd.dma_gather(
                    g_t[:], src_dram[:, :], idx_t[:],
                    num_idxs=n_in * 128, num_idxs_reg=n_in * 128, elem_size=D,
                    single_packet=False)
                gtiles[(si, grp)] = g_t
            return gtiles[(si, grp)][:, i % GCH, :]

        for i in range(nchunks):
            ps_lin = self.p_ps2.tile([128, Dout], f32, tag="lin")
            for si, (src, idx_dram) in enumerate(sources):
                if idx_dram is None:
                    chk = self.p_lin.tile([128, D], bf16, tag="lch")
                    nc.sync.dma_start(chk[:], src[128 * i:128 * (i + 1), :])
                    src_view = chk[:]
                else:
                    src_view = gathered_view(si, i, src, idx_dram)
                ps_t = self.p_ps2.tile([128, D], bf16, tag="tp")
                nc.tensor.transpose(ps_t[:], src_view, self.ident_bf[:])
                tt = self.p_lin.tile([128, D], bf16, tag="ltt")
                nc.vector.tensor_copy(tt[:], ps_t[:])
                nc.tensor.matmul(ps_lin[:], tt[:], Ws[si][:],
                                 start=(si == 0), stop=(si == len(sources) - 1))
            outt = self.p_lin.tile([128, Dout], dst_dram.dtype, tag="lout")
            if bias_t is not None:
                bsum = self.p_lin.tile([128, Dout], f32, tag="lbs")
                nc.vector.tensor_add(bsum[:], ps_lin[:], bias_t[:])
                src_ap = bsum[:]
            else:
                src_ap = ps_lin[:]
            nc.scalar.activation(
                outt[:], src_ap, func_copy,
                scale=(scale_t[:, i:i + 1] if scale_t is not None else 1.0))
            nc.sync.dma_start(dst_dram[128 * i:128 * (i + 1), :], outt[:])


# ---------------------------------------------------------------- main
def build(inputs, nphases=999, do_compile=True):
    import concourse.bass as bass  # noqa: F401
    import concourse.tile as tile
    from concourse import bacc, mybir
    from contextlib import ExitStack

    X = np.ascontiguousarray(inputs["X"], np.float32)
    H = [
        (np.asarray(inputs["H0_v"]).astype(np.int64),
         np.asarray(inputs["H0_e"]).astype(np.int64), N0, E0),
        (np.asarray(inputs["H1_v"]).astype(np.int64),
         np.asarray(inputs["H1_e"]).astype(np.int64), N1, E1),
        (np.asarray(inputs["H2_v"]).astype(np.int64),
         np.asarray(inputs["H2_e"]).astype(np.int64), N2, E2),
    ]
    assign0 = np.asarray(inputs["assign0"]).astype(np.int64)
    assign1 = np.asarray(inputs["assign1"]).astype(np.int64)

    n0l, n1l, n2l = _pad_local(N0), _pad_local(N1), _pad_local(N2)
    e0p, e1p, e2p = _pad128(E0), _pad128(E1), _pad128(E2)

    dv_planes, de_planes = [], []

    def lap_streams(lv, nloc_pad):
        vi, ei, n, e = H[lv]
        ei = _balance_edges(vi, ei, e)[ei]
        dv_is, de_i = _degree_weights(vi, ei, n, e)
        owner, slot = vi % C, vi // C
        s1g, s1t, s2g, s2t = [], [], [], []
        for c in range(C):
            m = owner == c
            s1g.append(slot[m])
            s1t.append(ei[m])
            s2g.append(ei[m])
            s2t.append(slot[m])
        st1 = SegStage(f"l{lv}s1", s1g, s1t, _pad128(e))
        st2 = SegStage(f"l{lv}s2", s2g, s2t, nloc_pad)
        # per-core dv plane over local slots; de plane over edge windows
        dvp = []
        for c in range(C):
            loc = np.zeros(nloc_pad, np.float32)
            ids = np.arange(c, n, C)
            loc[: len(ids)] = dv_is[ids]
            dvp.append(loc.reshape(-1, 128).T.copy())
        dep = np.zeros(_pad128(e), np.float32)
        dep[:e] = de_i
        dep = dep.reshape(-1, 128).T.copy()
        dv_planes.append(dvp)
        de_planes.append(dep)
        return st1, st2

    st1_0, st2_0 = lap_streams(0, n0l)
    st1_1, st2_1 = lap_streams(1, n1l)
    st1_2, st2_2 = lap_streams(2, n2l)

    def pool_streams(name, assign, nfine, ncoarse, ncl_pad):
        cnt = np.bincount(assign, minlength=ncoarse).astype(np.float32)
        inv = np.where(cnt > 0, 1.0 / cnt, 0.0).astype(np.float32)
        g = np.arange(nfine)
        owner, slot = g % C, g // C
        rows = (assign % C) * ncl_pad + assign // C
        gi, tr = [], []
        for c in range(C):
            m = owner == c
            gi.append(slot[m])
            tr.append(rows[m])
        st = SegStage(name, gi, tr, C * ncl_pad)
        # inverse-count scale plane over C*ncl_pad target rows (shared)
        full = np.zeros(C * ncl_pad, np.float32)
        rows_all = (np.arange(ncoarse) % C) * ncl_pad + np.arange(ncoarse) // C
        full[rows_all] = inv
        icp = full.reshape(-1, 128).T.copy()
        return st, icp

    pool0, ic1p = pool_streams("pool0", assign0, N0, N1, n1l)
    pool1, ic2p = pool_streams("pool1", assign1, N1, N2, n2l)

    def unpool_planes(assign, nfine, ncl_pad):
        idxs = []
        for c in range(C):
            a = assign[np.arange(c, nfine, C)]
            idxs.append((a % C) * ncl_pad + a // C)
        return _gather_planes(idxs)

    up1_L, up1_planes = unpool_planes(assign1, N1, n2l)
    up0_L, up0_planes = unpool_planes(assign0, N0, n1l)

    nc = bacc.Bacc("TRN2", target_bir_lowering=False, debug=False,
                   num_devices=C)
    f32, i16 = mybir.dt.float32, mybir.dt.int16
    bf16 = mybir.dt.bfloat16
    B = Builder(nc, mybir)

    x_arrs = []
    for c in range(C):
        xc = X[c::C]
        xc = np.vstack([xc, np.zeros((n0l - len(xc), D_IN), np.float32)])
        x_arrs.append(xc.astype(BF16))
    x_d = B.add_input("x", [n0l, D_IN], bf16, x_arrs)
    out_d = nc.dram_tensor("out", [n0l, D_OUT], f32, kind="ExternalOutput")

    def dram(name, rows, d, dt=bf16, shared=False):
        return nc.dram_tensor(name, [rows, d], dt,
                              addr_space="Shared" if shared else "Local")

    T0 = dram("T0", n0l, D_H)
    Y0p, Y0f = dram("Y0p", e0p, D_H), dram("Y0f", e0p, D_H, shared=True)
    h0 = dram("h0", n0l, D_H)
    P1p, P1s = dram("P1p", C * n1l, D_H), dram("P1s", n1l, D_H)
    T1 = dram("T1", n1l, D_H)
    Y1p, Y1f = dram("Y1p", e1p, D_H), dram("Y1f", e1p, D_H, shared=True)
    h1 = dram("h1", n1l, D_H)
    P2p, P2s = dram("P2p", C * n2l, D_H), dram("P2s", n2l, D_H)
    T2 = dram("T2", n2l, D_H)
    Y2p, Y2f = dram("Y2p", e2p, D_H), dram("Y2f", e2p, D_H, shared=True)
    Xc2, Xc2f = dram("Xc2", n2l, D_H), dram("Xc2f", C * n2l, D_H, shared=True)
    T3 = dram("T3", n1l, D_H)
    Y3p, Y3f = dram("Y3p", e1p, D_H), dram("Y3f", e1p, D_H, shared=True)
    Xu1, Xuf = dram("Xu1", n1l, D_H), dram("Xuf", C * n1l, D_H, shared=True)
    T4 = dram("T4", n0l, D_OUT, dt=f32)
    Y4p = dram("Y4p", e0p, D_OUT, dt=f32)
    Y4f = dram("Y4f", e0p, D_OUT, dt=f32, shared=True)

    up1_d = B.add_input("up1_idx", [128, up1_L // 16], i16, up1_planes)
    up0_d = B.add_input("up0_idx", [128, up0_L // 16], i16, up0_planes)

    rg = [list(range(C))]

    def AR(src, dst, nchunks=1, nw=None):
        if nchunks == 1:
            nc.gpsimd.collective_compute(
                "AllReduce", mybir.AluOpType.add, replica_groups=rg,
                ins=[src.ap().opt()], outs=[dst.ap().opt()])
            return
        bounds = [round(i * nw / nchunks) * 128 for i in range(nchunks + 1)]
        for a, b in zip(bounds, bounds[1:]):
            nc.gpsimd.collective_compute(
                "AllReduce", mybir.AluOpType.add, replica_groups=rg,
                ins=[src[a:b, :].opt()], outs=[dst[a:b, :].opt()])

    def RS(src, dst):
        nc.gpsimd.collective_compute(
            "ReduceScatter", mybir.AluOpType.add, replica_groups=rg,
            ins=[src.ap().opt()], outs=[dst.ap().opt()])

    def AG(src, dst):
        nc.gpsimd.collective_compute(
            "AllGather", mybir.AluOpType.bypass, replica_groups=rg,
            ins=[src.ap().opt()], outs=[dst.ap().opt()])

    with ExitStack() as ctx:
        tc = ctx.enter_context(tile.TileContext(nc))
        B.tc = tc
        B.setup_pools(ctx)
        B.setup_consts()
        W0t = B.const_mat("w0", np.asarray(inputs["W0"]), bf16)
        W1t = B.const_mat("w1m", np.asarray(inputs["W1"]), bf16)
        W2t = B.const_mat("w2m", np.asarray(inputs["W2"]), bf16)
        W3a = B.const_mat("w3a", np.asarray(inputs["W3"])[:128], bf16)
        W3b = B.const_mat("w3b", np.asarray(inputs["W3"])[128:], bf16)
        W4a = B.const_mat("w4a", np.asarray(inputs["W4"])[:128], bf16)
        W4b = B.const_mat("w4b", np.asarray(inputs["W4"])[128:], bf16)
        dv0 = B.add_input("dv0p", [128, n0l // 128], f32, dv_planes[0])
        dv1 = B.add_input("dv1p", [128, n1l // 128], f32, dv_planes[1])
        dv2 = B.add_input("dv2p", [128, n2l // 128], f32, dv_planes[2])
        de0 = B.add_input("de0p", [128, e0p // 128], f32, de_planes[0])
        de1 = B.add_input("de1p", [128, e1p // 128], f32, de_planes[1])
        de2 = B.add_input("de2p", [128, e2p // 128], f32, de_planes[2])
        ic1 = B.add_input("ic1p", [128, C * n1l // 128], f32, ic1p)
        ic2 = B.add_input("ic2p", [128, C * n2l // 128], f32, ic2p)

        def load_plane(d, cols, tag):
            t = B.p_const.tile([128, cols], f32, tag=tag)
            nc.sync.dma_start(t[:], d[:, :])
            return t

        dv0t = load_plane(dv0, n0l // 128, "dv0t")
        dv1t = load_plane(dv1, n1l // 128, "dv1t")
        dv2t = load_plane(dv2, n2l // 128, "dv2t")
        de0t = load_plane(de0, e0p // 128, "de0t")
        de1t = load_plane(de1, e1p // 128, "de1t")
        de2t = load_plane(de2, e2p // 128, "de2t")
        ic1t = load_plane(ic1, C * n1l // 128, "ic1t")
        ic2t = load_plane(ic2, C * n2l // 128, "ic2t")

        ins = inputs
        phases = [
            lambda: B.emit_linear("lin0", [(x_d, None)], [W0t], ins["b0"], T0,
                                  n0l // 128, D_H, scale_t=dv0t),
            lambda: B.emit_seg(st1_0, T0, Y0p, D_H, "copy", de0t),
            lambda: AR(Y0p, Y0f, 4, e0p // 128),
            lambda: B.emit_seg(st2_0, Y0f, h0, D_H, "relu", dv0t),
            lambda: B.emit_seg(pool0, h0, P1p, D_H, "copy", ic1t),
            lambda: RS(P1p, P1s),
            lambda: B.emit_linear("lin1", [(P1s, None)], [W1t], ins["b1"], T1,
                                  n1l // 128, D_H, scale_t=dv1t),
            lambda: B.emit_seg(st1_1, T1, Y1p, D_H, "copy", de1t),
            lambda: AR(Y1p, Y1f, 2, e1p // 128),
            lambda: B.emit_seg(st2_1, Y1f, h1, D_H, "relu", dv1t),
            lambda: B.emit_seg(pool1, h1, P2p, D_H, "copy", ic2t),
            lambda: RS(P2p, P2s),
            lambda: B.emit_linear("lin2", [(P2s, None)], [W2t], ins["b2"], T2,
                                  n2l // 128, D_H, scale_t=dv2t),
            lambda: B.emit_seg(st1_2, T2, Y2p, D_H, "copy", de2t),
            lambda: AR(Y2p, Y2f),
            lambda: B.emit_seg(st2_2, Y2f, Xc2, D_H, "relu", dv2t),
            lambda: AG(Xc2, Xc2f),
            lambda: B.emit_linear("lin3", [(Xc2f, up1_d), (h1, None)],
                                  [W3a, W3b], ins["b3"], T3, n1l // 128, D_H,
                                  scale_t=dv1t),
            lambda: B.emit_seg(st1_1.renamed("l1bs1"), T3, Y3p, D_H, "copy",
                               de1t),
            lambda: AR(Y3p, Y3f, 2, e1p // 128),
            lambda: B.emit_seg(st2_1.renamed("l1bs2"), Y3f, Xu1, D_H, "relu",
                               dv1t),
            lambda: AG(Xu1, Xuf),
            lambda: B.emit_linear("lin4", [(Xuf, up0_d), (h0, None)],
                                  [W4a, W4b], ins["b4"], T4, n0l // 128, D_OUT,
                                  scale_t=dv0t),
            lambda: B.emit_seg(st1_0.renamed("l0bs1"), T4, Y4p, D_OUT, "copy",
                               de0t, fp32=True),
            lambda: AR(Y4p, Y4f, 4, e0p // 128),
            lambda: B.emit_seg(st2_0.renamed("l0bs2"), Y4f, out_d, D_OUT,
                               "copy", dv0t, fp32=True),
        ]
        for ph in phases[:nphases]:
            ph()
    if do_compile:
        nc.compile()

    in_maps = []
    for c in range(C):
        m = {}
        for name, arrs in B.inputs.items():
            m[name] = arrs[c] if isinstance(arrs, list) else arrs
        in_maps.append(m)
    return nc, in_maps


LAST_EXEC_NS = None


def _install_ntff_hook():
    import contextlib, ctypes, os, types
    try:
        from antenv import axon_hooks  # noqa: F401
        return
    except ImportError:
        pass
    import antenv
    so_path = os.environ.get("PJRT_LIBRARY_PATH", "/opt/axon/libaxon_pjrt.so")
    try:
        lib = ctypes.CDLL(so_path)
    except OSError:
        lib = None
    hook = None
    if lib is not None and hasattr(lib, "axon_start_nrt_profile"):
        lib.axon_start_nrt_profile.argtypes = [
            ctypes.POINTER(ctypes.c_int64), ctypes.c_size_t]
        lib.axon_start_nrt_profile.restype = ctypes.c_int64
        lib.axon_stop_nrt_profile.argtypes = [ctypes.c_char_p]
        lib.axon_stop_nrt_profile.restype = ctypes.c_int64

        @contextlib.contextmanager
        def hook(output_dir, device_ids):
            import jax
            jax.devices()
            if device_ids:
                ids = (ctypes.c_int64 * len(device_ids))(*device_ids)
                rc = lib.axon_start_nrt_profile(ids, len(device_ids))
            else:
                rc = lib.axon_start_nrt_profile(None, 0)
            if rc != 0:
                raise RuntimeError(f"axon_start_nrt_profile rc={rc}")
            try:
                yield
            finally:
                lib.axon_stop_nrt_profile(str(output_dir).encode())

    mod = types.ModuleType("antenv.axon_hooks")
    mod._hook = hook
    mod.get_axon_ntff_profile_hook = lambda: mod._hook
    def _set(h):
        mod._hook = h
    mod.set_axon_ntff_profile_hook = _set
    sys.modules["antenv.axon_hooks"] = mod
    antenv.axon_hooks = mod


def kernel(**inputs):
    global LAST_EXEC_NS
    import os
    trace = os.environ.get("HGNN_TRACE", "0") == "1"
    if trace:
        _install_ntff_hook()
    nc, in_maps = build(inputs)
    from concourse.bass_utils import run_bass_kernel_spmd
    res = run_bass_kernel_spmd(nc, in_maps, core_ids=list(range(C)),
                               trace=trace)
    LAST_EXEC_NS = res.exec_time_ns
    out = np.empty((N0, D_OUT), np.float32)
    for c in range(C):
        n = len(range(c, N0, C))
        out[c::C] = res.results[c]["out"][:n]
    return out


# revision 12
# speedup vs baseline: 1.0946x; 1.0107x over previous
"""Trainium2 Bass kernel for 3-level hierarchical hypergraph GNN (HGNN).

Strategy (8 NeuronCores, one SPMD NEFF, per-core index data):
  - Nodes of every level sharded round-robin: global id g -> core g%8, slot g//8.
  - Incidence entries assigned to the owner core of their node endpoint.
  - L_apply = two segment-sum passes:
      stage1 (edges): partial edge sums via dma_gather(node rows) + BINARY
                      one-hot matmul into 128-row PSUM edge windows; AllReduce.
      stage2 (nodes): gather full-edge-table rows + binary one-hot matmul into
                      local node windows (complete rows, no reduction needed).
    Degree scalings (Dv^-1/2, De^-1) are folded into per-row scales applied on
    the Activation engine (linear outputs x dv, stage1 outputs x de, stage2
    outputs x dv), so one-hots stay exactly {0,1}.
  - One-hots for a whole gather chunk are built with a single DVE
    tensor_tensor is_equal using stride-0 broadcast APs (iota vs target cols),
    avoiding the pathologically slow per-block tensor_scalar path.
  - bf16 tables + bf16 matmuls everywhere except the final D=64 L_apply
    (fp32; dma_gather requires elem >= 256B).
  - Pools/unpools: same machinery; ReduceScatter / AllGather for cluster maps.
"""
import sys

sys.path.insert(0, "/opt/trn_rl_repo")
import numpy as np
import ml_dtypes

BF16 = ml_dtypes.bfloat16

C = 8
CH = 64  # gather chunk size in 128-entry blocks (bf16 stages)
CHF = 32  # chunk size for fp32 stages

N0, N1, N2 = 100000, 25000, 6250
E0, E1, E2 = 20000, 5000, 1250
D_IN, D_H, D_OUT = 128, 128, 64


def _pad128(n):
    return ((n + 127) // 128) * 128


def _pad_local(n):
    return _pad128(-(-n // C))


# ---------------------------------------------------------------- host side
def _balance_edges(vi, ei, e):
    """Renumber edges so per-(core,window) stage-1 entry counts are balanced.

    Returns perm with new_id = perm[old_id]."""
    ep = _pad128(e)
    nw = ep // 128
    d = np.zeros((C, e), np.int64)
    for c in range(C):
        m = (vi % C) == c
        d[c] = np.bincount(ei[m], minlength=e)
    order = np.argsort(-d.sum(axis=0), kind="stable")
    load = np.zeros((C, nw), np.int64)
    cap = np.full(nw, 128, np.int64)
    cap[nw - 1] = 128 - (ep - e)
    perm = np.empty(e, np.int64)
    slot_next = np.zeros(nw, np.int64)
    for eid in order:
        cand = cap > 0
        score = (load[:, cand] + d[:, eid][:, None]).max(axis=0)
        w = np.nonzero(cand)[0][np.argmin(score)]
        load[:, w] += d[:, eid]
        perm[eid] = w * 128 + slot_next[w]
        slot_next[w] += 1
        cap[w] -= 1
    return perm


def _degree_weights(vi, ei, n, e):
    ones = np.ones(len(vi), np.float32)
    dV = np.bincount(vi, weights=ones, minlength=n)
    dE = np.bincount(ei, weights=ones, minlength=e)
    dv_is = np.where(dV > 0, dV ** -0.5, 0.0).astype(np.float32)
    de_i = np.where(dE > 0, 1.0 / dE, 0.0).astype(np.float32)
    return dv_is, de_i


def _plane_idx(idx):
    """int array (L,) -> [128, L//16] int16 (16-partition wrap, replicated x8)."""
    assert len(idx) % 16 == 0
    assert idx.max(initial=0) < 32768
    return np.tile(idx.astype(np.int16).reshape(-1, 16).T, (C, 1)).copy()


def _plane_tgt(tgt, valid):
    """-> [128, B] target col per (lane, block); -1 where padded."""
    nb = len(tgt) // 128
    t = np.where(valid, tgt.astype(np.float32), -1.0).astype(np.float32)
    return t.reshape(nb, 128).T.copy()


class SegStage:
    """Host data for one segment-sum stage, uniform structure across cores."""

    def __init__(self, name, gidx, trow, n_rows_padded):
        self.name = name
        self.nw = n_rows_padded // 128
        cnts = np.stack([
            np.bincount(trow[c] // 128, minlength=self.nw) for c in range(C)
        ])
        self.bpw = np.maximum(1, -(-cnts.max(axis=0) // 128)).astype(np.int64)
        self.nblocks = int(self.bpw.sum())
        L = 128 * self.nblocks
        self.L = L
        self.idx_planes, self.tgt_planes = [], []
        starts = np.concatenate([[0], np.cumsum(self.bpw[:-1])]) * 128
        for c in range(C):
            order = np.argsort(trow[c] // 128, kind="stable")
            gi = np.zeros(L, np.int64)
            tg = np.zeros(L, np.int64)
            va = np.zeros(L, bool)
            w_of = trow[c] // 128
            pos = np.searchsorted(w_of[order], np.arange(self.nw))
            end = np.searchsorted(w_of[order], np.arange(self.nw), side="right")
            for w in range(self.nw):
                sel = order[pos[w]:end[w]]
                s = starts[w]
                gi[s:s + len(sel)] = gidx[c][sel]
                tg[s:s + len(sel)] = trow[c][sel] % 128
                va[s:s + len(sel)] = True
            self.idx_planes.append(_plane_idx(gi))
            self.tgt_planes.append(_plane_tgt(tg, va))

    def renamed(self, name):
        st = SegStage.__new__(SegStage)
        st.__dict__ = dict(self.__dict__)
        st.name = name
        return st


def _gather_planes(idx_per_core):
    """Plain gather streams (no reduction), padded to a 128 multiple."""
    L = _pad128(len(idx_per_core[0]))
    planes = []
    for c in range(C):
        gi = np.zeros(L, np.int64)
        gi[: len(idx_per_core[c])] = idx_per_core[c]
        planes.append(_plane_idx(gi))
    return L, planes


# ---------------------------------------------------------------- device side
class Builder:
    def __init__(self, nc, mybir):
        self.nc = nc
        self.mybir = mybir
        self.tc = None
        self.inputs = {}  # name -> per-core list of arrays (or one shared array)

    def add_input(self, name, shape, dtype, arrays):
        assert name not in self.inputs, name
        t = self.nc.dram_tensor(name, list(shape), dtype, kind="ExternalInput")
        self.inputs[name] = arrays
        return t

    def setup_pools(self, ctx):
        tc = self.tc
        self.p_const = ctx.enter_context(tc.tile_pool(name="const", bufs=1))
        self.p_gath = ctx.enter_context(tc.tile_pool(name="gath", bufs=3))
        self.p_meta = ctx.enter_context(tc.tile_pool(name="meta", bufs=4))
        self.p_oh = ctx.enter_context(tc.tile_pool(name="oh", bufs=2))
        self.p_fl = ctx.enter_context(tc.tile_pool(name="fl", bufs=6))
        self.p_lin = ctx.enter_context(tc.tile_pool(name="lin", bufs=3))
        self.p_ps = ctx.enter_context(tc.tile_pool(name="ps", bufs=4, space="PSUM"))
        self.p_ps2 = ctx.enter_context(tc.tile_pool(name="ps2", bufs=2, space="PSUM"))

    def setup_consts(self):
        f32 = self.mybir.dt.float32
        bf16 = self.mybir.dt.bfloat16
        iota = np.tile(np.arange(128, dtype=np.float32), (128, 1))
        ident = np.eye(128, dtype=np.float32)
        self.iota_bf = self.const_mat("c_iotab", iota, bf16)
        self.iota_f = self.const_mat("c_iotaf", iota, f32)
        self.ident_bf = self.const_mat("c_identb", ident, bf16)

    def const_mat(self, name, arr, dt=None):
        mybir = self.mybir
        dt = dt if dt is not None else mybir.dt.float32
        if dt == mybir.dt.bfloat16:
            arr = np.ascontiguousarray(arr.astype(np.float32)).astype(BF16)
        else:
            arr = np.ascontiguousarray(arr, np.float32)
        d = self.add_input(name, list(arr.shape), dt, arr)
        t = self.p_const.tile(list(arr.shape), dt, tag=name)
        self.nc.sync.dma_start(t[:], d[:, :])
        return t

    def emit_seg(self, st: SegStage, src_dram, dst_dram, D, out_op, scale_t,
                 fp32=False):
        """One segment-sum stage. out_op in ('copy', 'relu').

        scale_t: const tile [128, st.nw]; output window w is scaled per-row by
        scale_t[:, w] on the Activation engine.
        """
        nc, mybir = self.nc, self.mybir
        f32, i16 = mybir.dt.float32, mybir.dt.int16
        bf16 = mybir.dt.bfloat16
        dt = f32 if fp32 else bf16
        iota_t = self.iota_f if fp32 else self.iota_bf
        ch = CHF if fp32 else CH
        idx_d = self.add_input(f"{st.name}_idx", [128, st.L // 16], i16,
                               st.idx_planes)
        tgt_d = self.add_input(
            f"{st.name}_tg", [128, st.nblocks], dt,
            [p if fp32 else p.astype(BF16) for p in st.tgt_planes])
        func_relu = mybir.ActivationFunctionType.Relu
        func_copy = mybir.ActivationFunctionType.Copy

        sched = []  # block -> (window, j, is_last)
        for w in range(st.nw):
            for j in range(st.bpw[w]):
                sched.append((w, j, j == st.bpw[w] - 1))

        b = 0
        ps = None
        for start in range(0, st.nblocks, ch):
            nb = min(ch, st.nblocks - start)
            idx_t = self.p_meta.tile([128, nb * 8], i16, tag="idx")
            nc.scalar.dma_start(idx_t[:],
                                idx_d[:, start * 8:(start + nb) * 8])
            tgt_t = self.p_meta.tile([128, nb], dt, tag="tg")
            nc.scalar.dma_start(tgt_t[:], tgt_d[:, start:start + nb])
            g_t = self.p_gath.tile([128, nb, D], dt, tag=f"g{D}{dt}")
            nc.gpsimd.dma_gather(
                g_t[:], src_dram[:, :], idx_t[:],
                num_idxs=nb * 128, num_idxs_reg=nb * 128, elem_size=D,
                single_packet=False)
            oh_t = self.p_oh.tile([128, nb, 128], dt, tag=f"oh{dt}")
            nc.vector.tensor_tensor(
                oh_t[:],
                iota_t[:].unsqueeze(1).to_broadcast([128, nb, 128]),
                tgt_t[:].unsqueeze(2).to_broadcast([128, nb, 128]),
                mybir.AluOpType.is_equal)
            for k in range(nb):
                w, j, last = sched[b]
                if j == 0:
                    ps = self.p_ps.tile([128, D], f32, tag="seg")
                nc.tensor.matmul(ps[:], oh_t[:, k, :], g_t[:, k, :],
                                 start=(j == 0), stop=last)
                if last:
                    r = self.p_fl.tile([128, D], dst_dram.dtype, tag="fl")
                    nc.scalar.activation(
                        r[:], ps[:],
                        func_relu if out_op == "relu" else func_copy,
                        scale=scale_t[:, w:w + 1])
                    nc.sync.dma_start(dst_dram[128 * w:128 * (w + 1), :], r[:])
                b += 1

    def emit_linear(self, name, sources, Ws, bias, dst_dram, nchunks, Dout,
                    scale_t=None, D=128):
        """dst chunk = (sum_s source_s_chunk @ Ws[s] + bias) * scale_row.

        sources: list of (src_dram, None) for sequential 128-row chunks, or
        (src_dram, idx_dram) for rows gathered via a per-core index stream.
        bias: np vector or None. scale_t: const tile [128, nchunks] or None."""
        nc, mybir = self.nc, self.mybir
        f32, i16 = mybir.dt.float32, mybir.dt.int16
        bf16 = mybir.dt.bfloat16
        GCH = 16  # chunks per gather group
        gtiles = {}
        bias_t = None
        if bias is not None and np.any(np.asarray(bias) != 0):
            bias_t = self.const_mat(f"{name}_b", np.tile(bias, (128, 1)))
        func_copy = mybir.ActivationFunctionType.Copy

        def gathered_view(si, i, src_dram, idx_dram):
            grp = i // GCH
            if (si, grp) not in gtiles:
                n_in = min(GCH, nchunks - grp * GCH)
                idx_t = self.p_meta.tile([128, n_in * 8], i16, tag="lidx")
                nc.scalar.dma_start(
                    idx_t[:],
                    idx_dram[:, grp * GCH * 8:(grp * GCH + n_in) * 8])
                g_t = self.p_gath.tile([128, n_in, D], bf16, tag="lg")
                nc.gpsimd.dma_gather(
                    g_t[:], src_dram[:, :], idx_t[:],
                    num_idxs=n_in * 128, num_idxs_reg=n_in * 128, elem_size=D,
                    single_packet=False)
                gtiles[(si, grp)] = g_t
            return gtiles[(si, grp)][:, i % GCH, :]

        for i in range(nchunks):
            ps_lin = self.p_ps2.tile([128, Dout], f32, tag="lin")
            for si, (src, idx_dram) in enumerate(sources):
                if idx_dram is None:
                    chk = self.p_lin.tile([128, D], bf16, tag="lch")
                    nc.scalar.dma_start(chk[:], src[128 * i:128 * (i + 1), :])
                    src_view = chk[:]
                else:
                    src_view = gathered_view(si, i, src, idx_dram)
                ps_t = self.p_ps2.tile([128, D], bf16, tag="tp")
                nc.tensor.transpose(ps_t[:], src_view, self.ident_bf[:])
                tt = self.p_lin.tile([128, D], bf16, tag="ltt")
                nc.vector.tensor_copy(tt[:], ps_t[:])
                nc.tensor.matmul(ps_lin[:], tt[:], Ws[si][:],
                                 start=(si == 0), stop=(si == len(sources) - 1))
            outt = self.p_lin.tile([128, Dout], dst_dram.dtype, tag="lout")
            if bias_t is not None:
                bsum = self.p_lin.tile([128, Dout], f32, tag="lbs")
                nc.vector.tensor_add(bsum[:], ps_lin[:], bias_t[:])
                src_ap = bsum[:]
            else:
                src_ap = ps_lin[:]
            nc.scalar.activation(
                outt[:], src_ap, func_copy,
                scale=(scale_t[:, i:i + 1] if scale_t is not None else 1.0))
            nc.sync.dma_start(dst_dram[128 * i:128 * (i + 1), :], outt[:])


# ---------------------------------------------------------------- main
def build(inputs, nphases=999, do_compile=True):
    import concourse.bass as bass  # noqa: F401
    import concourse.tile as tile
    from concourse import bacc, mybir
    from contextlib import ExitStack

    X = np.ascontiguousarray(inputs["X"], np.float32)
    H = [
        (np.asarray(inputs["H0_v"]).astype(np.int64),
         np.asarray(inputs["H0_e"]).astype(np.int64), N0, E0),
        (np.asarray(inputs["H1_v"]).astype(np.int64),
         np.asarray(inputs["H1_e"]).astype(np.int64), N1, E1),
        (np.asarray(inputs["H2_v"]).astype(np.int64),
         np.asarray(inputs["H2_e"]).astype(np.int64), N2, E2),
    ]
    assign0 = np.asarray(inputs["assign0"]).astype(np.int64)
    assign1 = np.asarray(inputs["assign1"]).astype(np.int64)

    n0l, n1l, n2l = _pad_local(N0), _pad_local(N1), _pad_local(N2)
    e0p, e1p, e2p = _pad128(E0), _pad128(E1), _pad128(E2)

    dv_planes, de_planes = [], []

    def lap_streams(lv, nloc_pad):
        vi, ei, n, e = H[lv]
        ei = _balance_edges(vi, ei, e)[ei]
        dv_is, de_i = _degree_weights(vi, ei, n, e)
        owner, slot = vi % C, vi // C
        s1g, s1t, s2g, s2t = [], [], [], []
        for c in range(C):
            m = owner == c
            s1g.append(slot[m])
            s1t.append(ei[m])
            s2g.append(ei[m])
            s2t.append(slot[m])
        st1 = SegStage(f"l{lv}s1", s1g, s1t, _pad128(e))
        st2 = SegStage(f"l{lv}s2", s2g, s2t, nloc_pad)
        # per-core dv plane over local slots; de plane over edge windows
        dvp = []
        for c in range(C):
            loc = np.zeros(nloc_pad, np.float32)
            ids = np.arange(c, n, C)
            loc[: len(ids)] = dv_is[ids]
            dvp.append(loc.reshape(-1, 128).T.copy())
        dep = np.zeros(_pad128(e), np.float32)
        dep[:e] = de_i
        dep = dep.reshape(-1, 128).T.copy()
        dv_planes.append(dvp)
        de_planes.append(dep)
        return st1, st2

    st1_0, st2_0 = lap_streams(0, n0l)
    st1_1, st2_1 = lap_streams(1, n1l)
    st1_2, st2_2 = lap_streams(2, n2l)

    def pool_streams(name, assign, nfine, ncoarse, ncl_pad):
        cnt = np.bincount(assign, minlength=ncoarse).astype(np.float32)
        inv = np.where(cnt > 0, 1.0 / cnt, 0.0).astype(np.float32)
        g = np.arange(nfine)
        owner, slot = g % C, g // C
        rows = (assign % C) * ncl_pad + assign // C
        gi, tr = [], []
        for c in range(C):
            m = owner == c
            gi.append(slot[m])
            tr.append(rows[m])
        st = SegStage(name, gi, tr, C * ncl_pad)
        # inverse-count scale plane over C*ncl_pad target rows (shared)
        full = np.zeros(C * ncl_pad, np.float32)
        rows_all = (np.arange(ncoarse) % C) * ncl_pad + np.arange(ncoarse) // C
        full[rows_all] = inv
        icp = full.reshape(-1, 128).T.copy()
        return st, icp

    pool0, ic1p = pool_streams("pool0", assign0, N0, N1, n1l)
    pool1, ic2p = pool_streams("pool1", assign1, N1, N2, n2l)

    def unpool_planes(assign, nfine, ncl_pad):
        idxs = []
        for c in range(C):
            a = assign[np.arange(c, nfine, C)]
            idxs.append((a % C) * ncl_pad + a // C)
        return _gather_planes(idxs)

    up1_L, up1_planes = unpool_planes(assign1, N1, n2l)
    up0_L, up0_planes = unpool_planes(assign0, N0, n1l)

    nc = bacc.Bacc("TRN2", target_bir_lowering=False, debug=False,
                   num_devices=C)
    f32, i16 = mybir.dt.float32, mybir.dt.int16
    bf16 = mybir.dt.bfloat16
    B = Builder(nc, mybir)

    x_arrs = []
    for c in range(C):
        xc = X[c::C]
        xc = np.vstack([xc, np.zeros((n0l - len(xc), D_IN), np.float32)])
        x_arrs.append(xc.astype(BF16))
    x_d = B.add_input("x", [n0l, D_IN], bf16, x_arrs)
    out_d = nc.dram_tensor("out", [n0l, D_OUT], f32, kind="ExternalOutput")

    def dram(name, rows, d, dt=bf16, shared=False):
        return nc.dram_tensor(name, [rows, d], dt,
                              addr_space="Shared" if shared else "Local")

    T0 = dram("T0", n0l, D_H)
    Y0p, Y0f = dram("Y0p", e0p, D_H), dram("Y0f", e0p, D_H, shared=True)
    h0 = dram("h0", n0l, D_H)
    P1p, P1s = dram("P1p", C * n1l, D_H), dram("P1s", n1l, D_H)
    T1 = dram("T1", n1l, D_H)
    Y1p, Y1f = dram("Y1p", e1p, D_H), dram("Y1f", e1p, D_H, shared=True)
    h1 = dram("h1", n1l, D_H)
    P2p, P2s = dram("P2p", C * n2l, D_H), dram("P2s", n2l, D_H)
    T2 = dram("T2", n2l, D_H)
    Y2p, Y2f = dram("Y2p", e2p, D_H), dram("Y2f", e2p, D_H, shared=True)
    Xc2, Xc2f = dram("Xc2", n2l, D_H), dram("Xc2f", C * n2l, D_H, shared=True)
    T3 = dram("T3", n1l, D_H)
    Y3p, Y3f = dram("Y3p", e1p, D_H), dram("Y3f", e1p, D_H, shared=True)
    Xu1, Xuf = dram("Xu1", n1l, D_H), dram("Xuf", C * n1l, D_H, shared=True)
    T4 = dram("T4", n0l, D_OUT, dt=f32)
    Y4p = dram("Y4p", e0p, D_OUT, dt=f32)
    Y4f = dram("Y4f", e0p, D_OUT, dt=f32, shared=True)

    up1_d = B.add_input("up1_idx", [128, up1_L // 16], i16, up1_planes)
    up0_d = B.add_input("up0_idx", [128, up0_L // 16], i16, up0_planes)

    rg = [list(range(C))]

    def AR(src, dst, nchunks=1, nw=None):
        if nchunks == 1:
            nc.gpsimd.collective_compute(
                "AllReduce", mybir.AluOpType.add, replica_groups=rg,
                ins=[src.ap().opt()], outs=[dst.ap().opt()])
            return
        bounds = [round(i * nw / nchunks) * 128 for i in range(nchunks + 1)]
        for a, b in zip(bounds, bounds[1:]):
            nc.gpsimd.collective_compute(
                "AllReduce", mybir.AluOpType.add, replica_groups=rg,
                ins=[src[a:b, :].opt()], outs=[dst[a:b, :].opt()])

    def RS(src, dst):
        nc.gpsimd.collective_compute(
            "ReduceScatter", mybir.AluOpType.add, replica_groups=rg,
            ins=[src.ap().opt()], outs=[dst.ap().opt()])

    def AG(src, dst):
        nc.gpsimd.collective_compute(
            "AllGather", mybir.AluOpType.bypass, replica_groups=rg,
            ins=[src.ap().opt()], outs=[dst.ap().opt()])

    with ExitStack() as ctx:
        tc = ctx.enter_context(tile.TileContext(nc))
        B.tc = tc
        B.setup_pools(ctx)
        B.setup_consts()
        W0t = B.const_mat("w0", np.asarray(inputs["W0"]), bf16)
        W1t = B.const_mat("w1m", np.asarray(inputs["W1"]), bf16)
        W2t = B.const_mat("w2m", np.asarray(inputs["W2"]), bf16)
        W3a = B.const_mat("w3a", np.asarray(inputs["W3"])[:128], bf16)
        W3b = B.const_mat("w3b", np.asarray(inputs["W3"])[128:], bf16)
        W4a = B.const_mat("w4a", np.asarray(inputs["W4"])[:128], bf16)
        W4b = B.const_mat("w4b", np.asarray(inputs["W4"])[128:], bf16)
        dv0 = B.add_input("dv0p", [128, n0l // 128], f32, dv_planes[0])
        dv1 = B.add_input("dv1p", [128, n1l // 128], f32, dv_planes[1])
        dv2 = B.add_input("dv2p", [128, n2l // 128], f32, dv_planes[2])
        de0 = B.add_input("de0p", [128, e0p // 128], f32, de_planes[0])
        de1 = B.add_input("de1p", [128, e1p // 128], f32, de_planes[1])
        de2 = B.add_input("de2p", [128, e2p // 128], f32, de_planes[2])
        ic1 = B.add_input("ic1p", [128, C * n1l // 128], f32, ic1p)
        ic2 = B.add_input("ic2p", [128, C * n2l // 128], f32, ic2p)

        def load_plane(d, cols, tag):
            t = B.p_const.tile([128, cols], f32, tag=tag)
            nc.sync.dma_start(t[:], d[:, :])
            return t

        dv0t = load_plane(dv0, n0l // 128, "dv0t")
        dv1t = load_plane(dv1, n1l // 128, "dv1t")
        dv2t = load_plane(dv2, n2l // 128, "dv2t")
        de0t = load_plane(de0, e0p // 128, "de0t")
        de1t = load_plane(de1, e1p // 128, "de1t")
        de2t = load_plane(de2, e2p // 128, "de2t")
        ic1t = load_plane(ic1, C * n1l // 128, "ic1t")
        ic2t = load_plane(ic2, C * n2l // 128, "ic2t")

        ins = inputs
        phases = [
            lambda: B.emit_linear("lin0", [(x_d, None)], [W0t], ins["b0"], T0,
                                  n0l // 128, D_H, scale_t=dv0t),
            lambda: B.emit_seg(st1_0, T0, Y0p, D_H, "copy", de0t),
            lambda: AR(Y0p, Y0f, 4, e0p // 128),
            lambda: B.emit_seg(st2_0, Y0f, h0, D_H, "relu", dv0t),
            lambda: B.emit_seg(pool0, h0, P1p, D_H, "copy", ic1t),
            lambda: RS(P1p, P1s),
            lambda: B.emit_linear("lin1", [(P1s, None)], [W1t], ins["b1"], T1,
                                  n1l // 128, D_H, scale_t=dv1t),
            lambda: B.emit_seg(st1_1, T1, Y1p, D_H, "copy", de1t),
            lambda: AR(Y1p, Y1f, 2, e1p // 128),
            lambda: B.emit_seg(st2_1, Y1f, h1, D_H, "relu", dv1t),
            lambda: B.emit_seg(pool1, h1, P2p, D_H, "copy", ic2t),
            lambda: RS(P2p, P2s),
            lambda: B.emit_linear("lin2", [(P2s, None)], [W2t], ins["b2"], T2,
                                  n2l // 128, D_H, scale_t=dv2t),
            lambda: B.emit_seg(st1_2, T2, Y2p, D_H, "copy", de2t),
            lambda: AR(Y2p, Y2f),
            lambda: B.emit_seg(st2_2, Y2f, Xc2, D_H, "relu", dv2t),
            lambda: AG(Xc2, Xc2f),
            lambda: B.emit_linear("lin3", [(Xc2f, up1_d), (h1, None)],
                                  [W3a, W3b], ins["b3"], T3, n1l // 128, D_H,
                                  scale_t=dv1t),
            lambda: B.emit_seg(st1_1.renamed("l1bs1"), T3, Y3p, D_H, "copy",
                               de1t),
            lambda: AR(Y3p, Y3f, 2, e1p // 128),
            lambda: B.emit_seg(st2_1.renamed("l1bs2"), Y3f, Xu1, D_H, "relu",
                               dv1t),
            lambda: AG(Xu1, Xuf),
            lambda: B.emit_linear("lin4", [(Xuf, up0_d), (h0, None)],
                                  [W4a, W4b], ins["b4"], T4, n0l // 128, D_OUT,
                                  scale_t=dv0t),
            lambda: B.emit_seg(st1_0.renamed("l0bs1"), T4, Y4p, D_OUT, "copy",
                               de0t, fp32=True),
            lambda: AR(Y4p, Y4f, 4, e0p // 128),
            lambda: B.emit_seg(st2_0.renamed("l0bs2"), Y4f, out_d, D_OUT,
                               "copy", dv0t, fp32=True),
        ]
        for ph in phases[:nphases]:
            ph()
    if do_compile:
        nc.compile()

    in_maps = []
    for c in range(C):
        m = {}
        for name, arrs in B.inputs.items():
            m[name] = arrs[c] if isinstance(arrs, list) else arrs
        in_maps.append(m)
    return nc, in_maps


LAST_EXEC_NS = None


def _install_ntff_hook():
    import contextlib, ctypes, os, types
    try:
        from antenv import axon_hooks  # noqa: F401
        return
    except ImportError:
        pass
    import antenv
    so_path = os.environ.get("PJRT_LIBRARY_PATH", "/opt/axon/libaxon_pjrt.so")
    try:
        lib = ctypes.CDLL(so_path)
    except OSError:
        lib = None
    hook = None
    if lib is not None and hasattr(lib, "axon_start_nrt_profile"):
        lib.axon_start_nrt_profile.argtypes = [
            ctypes.POINTER(ctypes.c_int64), ctypes.c_size_t]
        lib.axon_start_nrt_profile.restype = ctypes.c_int64
        lib.axon_stop_nrt_profile.argtypes = [ctypes.c_char_p]
        lib.axon_stop_nrt_profile.restype = ctypes.c_int64

        @contextlib.contextmanager
        def hook(output_dir, device_ids):
            import jax
            jax.devices()
            if device_ids:
                ids = (ctypes.c_int64 * len(device_ids))(*device_ids)
                rc = lib.axon_start_nrt_profile(ids, len(device_ids))
            else:
                rc = lib.axon_start_nrt_profile(None, 0)
            if rc != 0:
                raise RuntimeError(f"axon_start_nrt_profile rc={rc}")
            try:
                yield
            finally:
                lib.axon_stop_nrt_profile(str(output_dir).encode())

    mod = types.ModuleType("antenv.axon_hooks")
    mod._hook = hook
    mod.get_axon_ntff_profile_hook = lambda: mod._hook
    def _set(h):
        mod._hook = h
    mod.set_axon_ntff_profile_hook = _set
    sys.modules["antenv.axon_hooks"] = mod
    antenv.axon_hooks = mod


def kernel(**inputs):
    global LAST_EXEC_NS
    import os
    trace = os.environ.get("HGNN_TRACE", "0") == "1"
    if trace:
        _install_ntff_hook()
    nc, in_maps = build(inputs)
    from concourse.bass_utils import run_bass_kernel_spmd
    res = run_bass_kernel_spmd(nc, in_maps, core_ids=list(range(C)),
                               trace=trace)
    LAST_EXEC_NS = res.exec_time_ns
    out = np.empty((N0, D_OUT), np.float32)
    for c in range(C):
        n = len(range(c, N0, C))
        out[c::C] = res.results[c]["out"][:n]
    return out


# revision 13
# speedup vs baseline: 1.0961x; 1.0014x over previous
"""Trainium2 Bass kernel for 3-level hierarchical hypergraph GNN (HGNN).

Strategy (8 NeuronCores, one SPMD NEFF, per-core index data):
  - Nodes of every level sharded round-robin: global id g -> core g%8, slot g//8.
  - Incidence entries assigned to the owner core of their node endpoint.
  - L_apply = two segment-sum passes:
      stage1 (edges): partial edge sums via dma_gather(node rows) + BINARY
                      one-hot matmul into 128-row PSUM edge windows; AllReduce.
      stage2 (nodes): gather full-edge-table rows + binary one-hot matmul into
                      local node windows (complete rows, no reduction needed).
    Degree scalings (Dv^-1/2, De^-1) are folded into per-row scales applied on
    the Activation engine (linear outputs x dv, stage1 outputs x de, stage2
    outputs x dv), so one-hots stay exactly {0,1}.
  - One-hots for a whole gather chunk are built with a single DVE
    tensor_tensor is_equal using stride-0 broadcast APs (iota vs target cols),
    avoiding the pathologically slow per-block tensor_scalar path.
  - bf16 tables + bf16 matmuls everywhere except the final D=64 L_apply
    (fp32; dma_gather requires elem >= 256B).
  - Pools/unpools: same machinery; ReduceScatter / AllGather for cluster maps.
"""
import sys

sys.path.insert(0, "/opt/trn_rl_repo")
import numpy as np
import ml_dtypes

BF16 = ml_dtypes.bfloat16

C = 8
CH = 64  # gather chunk size in 128-entry blocks (bf16 stages)
CHF = 32  # chunk size for fp32 stages

N0, N1, N2 = 100000, 25000, 6250
E0, E1, E2 = 20000, 5000, 1250
D_IN, D_H, D_OUT = 128, 128, 64


def _pad128(n):
    return ((n + 127) // 128) * 128


def _pad_local(n):
    return _pad128(-(-n // C))


# ---------------------------------------------------------------- host side
def _balance_edges(vi, ei, e):
    """Renumber edges so per-(core,window) stage-1 entry counts are balanced.

    Returns perm with new_id = perm[old_id]."""
    ep = _pad128(e)
    nw = ep // 128
    d = np.zeros((C, e), np.int64)
    for c in range(C):
        m = (vi % C) == c
        d[c] = np.bincount(ei[m], minlength=e)
    order = np.argsort(-d.sum(axis=0), kind="stable")
    load = np.zeros((C, nw), np.int64)
    cap = np.full(nw, 128, np.int64)
    cap[nw - 1] = 128 - (ep - e)
    perm = np.empty(e, np.int64)
    slot_next = np.zeros(nw, np.int64)
    for eid in order:
        cand = cap > 0
        score = (load[:, cand] + d[:, eid][:, None]).max(axis=0)
        w = np.nonzero(cand)[0][np.argmin(score)]
        load[:, w] += d[:, eid]
        perm[eid] = w * 128 + slot_next[w]
        slot_next[w] += 1
        cap[w] -= 1
    return perm


def _degree_weights(vi, ei, n, e):
    ones = np.ones(len(vi), np.float32)
    dV = np.bincount(vi, weights=ones, minlength=n)
    dE = np.bincount(ei, weights=ones, minlength=e)
    dv_is = np.where(dV > 0, dV ** -0.5, 0.0).astype(np.float32)
    de_i = np.where(dE > 0, 1.0 / dE, 0.0).astype(np.float32)
    return dv_is, de_i


def _plane_idx(idx):
    """int array (L,) -> [128, L//16] int16 (16-partition wrap, replicated x8)."""
    assert len(idx) % 16 == 0
    assert idx.max(initial=0) < 32768
    return np.tile(idx.astype(np.int16).reshape(-1, 16).T, (C, 1)).copy()


def _plane_tgt(tgt, valid):
    """-> [128, B] target col per (lane, block); -1 where padded."""
    nb = len(tgt) // 128
    t = np.where(valid, tgt.astype(np.float32), -1.0).astype(np.float32)
    return t.reshape(nb, 128).T.copy()


class SegStage:
    """Host data for one segment-sum stage, uniform structure across cores."""

    def __init__(self, name, gidx, trow, n_rows_padded):
        self.name = name
        self.nw = n_rows_padded // 128
        cnts = np.stack([
            np.bincount(trow[c] // 128, minlength=self.nw) for c in range(C)
        ])
        self.bpw = np.maximum(1, -(-cnts.max(axis=0) // 128)).astype(np.int64)
        self.nblocks = int(self.bpw.sum())
        L = 128 * self.nblocks
        self.L = L
        self.idx_planes, self.tgt_planes = [], []
        starts = np.concatenate([[0], np.cumsum(self.bpw[:-1])]) * 128
        for c in range(C):
            order = np.argsort(trow[c] // 128, kind="stable")
            gi = np.zeros(L, np.int64)
            tg = np.zeros(L, np.int64)
            va = np.zeros(L, bool)
            w_of = trow[c] // 128
            pos = np.searchsorted(w_of[order], np.arange(self.nw))
            end = np.searchsorted(w_of[order], np.arange(self.nw), side="right")
            for w in range(self.nw):
                sel = order[pos[w]:end[w]]
                s = starts[w]
                gi[s:s + len(sel)] = gidx[c][sel]
                tg[s:s + len(sel)] = trow[c][sel] % 128
                va[s:s + len(sel)] = True
            self.idx_planes.append(_plane_idx(gi))
            self.tgt_planes.append(_plane_tgt(tg, va))

    def renamed(self, name):
        st = SegStage.__new__(SegStage)
        st.__dict__ = dict(self.__dict__)
        st.name = name
        return st


def _gather_planes(idx_per_core):
    """Plain gather streams (no reduction), padded to a 128 multiple."""
    L = _pad128(len(idx_per_core[0]))
    planes = []
    for c in range(C):
        gi = np.zeros(L, np.int64)
        gi[: len(idx_per_core[c])] = idx_per_core[c]
        planes.append(_plane_idx(gi))
    return L, planes


# ---------------------------------------------------------------- device side
class Builder:
    def __init__(self, nc, mybir):
        self.nc = nc
        self.mybir = mybir
        self.tc = None
        self.inputs = {}  # name -> per-core list of arrays (or one shared array)

    def add_input(self, name, shape, dtype, arrays):
        assert name not in self.inputs, name
        t = self.nc.dram_tensor(name, list(shape), dtype, kind="ExternalInput")
        self.inputs[name] = arrays
        return t

    def setup_pools(self, ctx):
        tc = self.tc
        self.p_const = ctx.enter_context(tc.tile_pool(name="const", bufs=1))
        self.p_gath = ctx.enter_context(tc.tile_pool(name="gath", bufs=3))
        self.p_meta = ctx.enter_context(tc.tile_pool(name="meta", bufs=4))
        self.p_oh = ctx.enter_context(tc.tile_pool(name="oh", bufs=2))
        self.p_fl = ctx.enter_context(tc.tile_pool(name="fl", bufs=6))
        self.p_lin = ctx.enter_context(tc.tile_pool(name="lin", bufs=3))
        self.p_ps = ctx.enter_context(tc.tile_pool(name="ps", bufs=4, space="PSUM"))
        self.p_ps2 = ctx.enter_context(tc.tile_pool(name="ps2", bufs=2, space="PSUM"))

    def setup_consts(self):
        f32 = self.mybir.dt.float32
        bf16 = self.mybir.dt.bfloat16
        iota = np.tile(np.arange(128, dtype=np.float32), (128, 1))
        ident = np.eye(128, dtype=np.float32)
        self.iota_bf = self.const_mat("c_iotab", iota, bf16)
        self.iota_f = self.const_mat("c_iotaf", iota, f32)
        self.ident_bf = self.const_mat("c_identb", ident, bf16)

    def const_mat(self, name, arr, dt=None):
        mybir = self.mybir
        dt = dt if dt is not None else mybir.dt.float32
        if dt == mybir.dt.bfloat16:
            arr = np.ascontiguousarray(arr.astype(np.float32)).astype(BF16)
        else:
            arr = np.ascontiguousarray(arr, np.float32)
        d = self.add_input(name, list(arr.shape), dt, arr)
        t = self.p_const.tile(list(arr.shape), dt, tag=name)
        self.nc.sync.dma_start(t[:], d[:, :])
        return t

    def emit_seg(self, st: SegStage, src_dram, dst_dram, D, out_op, scale_t,
                 fp32=False):
        """One segment-sum stage. out_op in ('copy', 'relu').

        scale_t: const tile [128, st.nw]; output window w is scaled per-row by
        scale_t[:, w] on the Activation engine.
        """
        nc, mybir = self.nc, self.mybir
        f32, i16 = mybir.dt.float32, mybir.dt.int16
        bf16 = mybir.dt.bfloat16
        dt = f32 if fp32 else bf16
        iota_t = self.iota_bf
        ch = CHF if fp32 else CH
        idx_d = self.add_input(f"{st.name}_idx", [128, st.L // 16], i16,
                               st.idx_planes)
        tgt_d = self.add_input(
            f"{st.name}_tg", [128, st.nblocks], bf16,
            [p.astype(BF16) for p in st.tgt_planes])
        func_relu = mybir.ActivationFunctionType.Relu
        func_copy = mybir.ActivationFunctionType.Copy

        sched = []  # block -> (window, j, is_last)
        for w in range(st.nw):
            for j in range(st.bpw[w]):
                sched.append((w, j, j == st.bpw[w] - 1))

        b = 0
        ps = None
        for start in range(0, st.nblocks, ch):
            nb = min(ch, st.nblocks - start)
            idx_t = self.p_meta.tile([128, nb * 8], i16, tag="idx")
            nc.scalar.dma_start(idx_t[:],
                                idx_d[:, start * 8:(start + nb) * 8])
            tgt_t = self.p_meta.tile([128, nb], bf16, tag="tg")
            nc.scalar.dma_start(tgt_t[:], tgt_d[:, start:start + nb])
            g_t = self.p_gath.tile([128, nb, D], dt, tag=f"g{D}{dt}")
            nc.gpsimd.dma_gather(
                g_t[:], src_dram[:, :], idx_t[:],
                num_idxs=nb * 128, num_idxs_reg=nb * 128, elem_size=D,
                single_packet=False)
            if fp32:
                gb_t = self.p_gath.tile([128, nb, D], bf16, tag=f"gb{D}")
                nc.vector.tensor_copy(gb_t[:], g_t[:])
            else:
                gb_t = g_t
            oh_t = self.p_oh.tile([128, nb, 128], bf16, tag="oh")
            nc.vector.tensor_tensor(
                oh_t[:],
                iota_t[:].unsqueeze(1).to_broadcast([128, nb, 128]),
                tgt_t[:].unsqueeze(2).to_broadcast([128, nb, 128]),
                mybir.AluOpType.is_equal)
            for k in range(nb):
                w, j, last = sched[b]
                if j == 0:
                    ps = self.p_ps.tile([128, D], f32, tag="seg")
                nc.tensor.matmul(ps[:], oh_t[:, k, :], gb_t[:, k, :],
                                 start=(j == 0), stop=last)
                if last:
                    r = self.p_fl.tile([128, D], dst_dram.dtype, tag="fl")
                    nc.scalar.activation(
                        r[:], ps[:],
                        func_relu if out_op == "relu" else func_copy,
                        scale=scale_t[:, w:w + 1])
                    nc.sync.dma_start(dst_dram[128 * w:128 * (w + 1), :], r[:])
                b += 1

    def emit_linear(self, name, sources, Ws, bias, dst_dram, nchunks, Dout,
                    scale_t=None, D=128):
        """dst chunk = (sum_s source_s_chunk @ Ws[s] + bias) * scale_row.

        sources: list of (src_dram, None) for sequential 128-row chunks, or
        (src_dram, idx_dram) for rows gathered via a per-core index stream.
        bias: np vector or None. scale_t: const tile [128, nchunks] or None."""
        nc, mybir = self.nc, self.mybir
        f32, i16 = mybir.dt.float32, mybir.dt.int16
        bf16 = mybir.dt.bfloat16
        GCH = 16  # chunks per gather group
        gtiles = {}
        bias_t = None
        if bias is not None and np.any(np.asarray(bias) != 0):
            bias_t = self.const_mat(f"{name}_b", np.tile(bias, (128, 1)))
        func_copy = mybir.ActivationFunctionType.Copy

        def gathered_view(si, i, src_dram, idx_dram):
            grp = i // GCH
            if (si, grp) not in gtiles:
                n_in = min(GCH, nchunks - grp * GCH)
                idx_t = self.p_meta.tile([128, n_in * 8], i16, tag="lidx")
                nc.scalar.dma_start(
                    idx_t[:],
                    idx_dram[:, grp * GCH * 8:(grp * GCH + n_in) * 8])
                g_t = self.p_gath.tile([128, n_in, D], bf16, tag="lg")
                nc.gpsimd.dma_gather(
                    g_t[:], src_dram[:, :], idx_t[:],
                    num_idxs=n_in * 128, num_idxs_reg=n_in * 128, elem_size=D,
                    single_packet=False)
                gtiles[(si, grp)] = g_t
            return gtiles[(si, grp)][:, i % GCH, :]

        for i in range(nchunks):
            ps_lin = self.p_ps2.tile([128, Dout], f32, tag="lin")
            for si, (src, idx_dram) in enumerate(sources):
                if idx_dram is None:
                    chk = self.p_lin.tile([128, D], bf16, tag="lch")
                    nc.scalar.dma_start(chk[:], src[128 * i:128 * (i + 1), :])
                    src_view = chk[:]
                else:
                    src_view = gathered_view(si, i, src, idx_dram)
                ps_t = self.p_ps2.tile([128, D], bf16, tag="tp")
                nc.tensor.transpose(ps_t[:], src_view, self.ident_bf[:])
                tt = self.p_lin.tile([128, D], bf16, tag="ltt")
                nc.vector.tensor_copy(tt[:], ps_t[:])
                nc.tensor.matmul(ps_lin[:], tt[:], Ws[si][:],
                                 start=(si == 0), stop=(si == len(sources) - 1))
            outt = self.p_lin.tile([128, Dout], dst_dram.dtype, tag="lout")
            if bias_t is not None:
                bsum = self.p_lin.tile([128, Dout], f32, tag="lbs")
                nc.vector.tensor_add(bsum[:], ps_lin[:], bias_t[:])
                src_ap = bsum[:]
            else:
                src_ap = ps_lin[:]
            nc.scalar.activation(
                outt[:], src_ap, func_copy,
                scale=(scale_t[:, i:i + 1] if scale_t is not None else 1.0))
            nc.sync.dma_start(dst_dram[128 * i:128 * (i + 1), :], outt[:])


# ---------------------------------------------------------------- main
def build(inputs, nphases=999, do_compile=True):
    import concourse.bass as bass  # noqa: F401
    import concourse.tile as tile
    from concourse import bacc, mybir
    from contextlib import ExitStack

    X = np.ascontiguousarray(inputs["X"], np.float32)
    H = [
        (np.asarray(inputs["H0_v"]).astype(np.int64),
         np.asarray(inputs["H0_e"]).astype(np.int64), N0, E0),
        (np.asarray(inputs["H1_v"]).astype(np.int64),
         np.asarray(inputs["H1_e"]).astype(np.int64), N1, E1),
        (np.asarray(inputs["H2_v"]).astype(np.int64),
         np.asarray(inputs["H2_e"]).astype(np.int64), N2, E2),
    ]
    assign0 = np.asarray(inputs["assign0"]).astype(np.int64)
    assign1 = np.asarray(inputs["assign1"]).astype(np.int64)

    n0l, n1l, n2l = _pad_local(N0), _pad_local(N1), _pad_local(N2)
    e0p, e1p, e2p = _pad128(E0), _pad128(E1), _pad128(E2)

    dv_planes, de_planes = [], []

    def lap_streams(lv, nloc_pad):
        vi, ei, n, e = H[lv]
        ei = _balance_edges(vi, ei, e)[ei]
        dv_is, de_i = _degree_weights(vi, ei, n, e)
        owner, slot = vi % C, vi // C
        s1g, s1t, s2g, s2t = [], [], [], []
        for c in range(C):
            m = owner == c
            s1g.append(slot[m])
            s1t.append(ei[m])
            s2g.append(ei[m])
            s2t.append(slot[m])
        st1 = SegStage(f"l{lv}s1", s1g, s1t, _pad128(e))
        st2 = SegStage(f"l{lv}s2", s2g, s2t, nloc_pad)
        # per-core dv plane over local slots; de plane over edge windows
        dvp = []
        for c in range(C):
            loc = np.zeros(nloc_pad, np.float32)
            ids = np.arange(c, n, C)
            loc[: len(ids)] = dv_is[ids]
            dvp.append(loc.reshape(-1, 128).T.copy())
        dep = np.zeros(_pad128(e), np.float32)
        dep[:e] = de_i
        dep = dep.reshape(-1, 128).T.copy()
        dv_planes.append(dvp)
        de_planes.append(dep)
        return st1, st2

    st1_0, st2_0 = lap_streams(0, n0l)
    st1_1, st2_1 = lap_streams(1, n1l)
    st1_2, st2_2 = lap_streams(2, n2l)

    def pool_streams(name, assign, nfine, ncoarse, ncl_pad):
        cnt = np.bincount(assign, minlength=ncoarse).astype(np.float32)
        inv = np.where(cnt > 0, 1.0 / cnt, 0.0).astype(np.float32)
        g = np.arange(nfine)
        owner, slot = g % C, g // C
        rows = (assign % C) * ncl_pad + assign // C
        gi, tr = [], []
        for c in range(C):
            m = owner == c
            gi.append(slot[m])
            tr.append(rows[m])
        st = SegStage(name, gi, tr, C * ncl_pad)
        # inverse-count scale plane over C*ncl_pad target rows (shared)
        full = np.zeros(C * ncl_pad, np.float32)
        rows_all = (np.arange(ncoarse) % C) * ncl_pad + np.arange(ncoarse) // C
        full[rows_all] = inv
        icp = full.reshape(-1, 128).T.copy()
        return st, icp

    pool0, ic1p = pool_streams("pool0", assign0, N0, N1, n1l)
    pool1, ic2p = pool_streams("pool1", assign1, N1, N2, n2l)

    def unpool_planes(assign, nfine, ncl_pad):
        idxs = []
        for c in range(C):
            a = assign[np.arange(c, nfine, C)]
            idxs.append((a % C) * ncl_pad + a // C)
        return _gather_planes(idxs)

    up1_L, up1_planes = unpool_planes(assign1, N1, n2l)
    up0_L, up0_planes = unpool_planes(assign0, N0, n1l)

    nc = bacc.Bacc("TRN2", target_bir_lowering=False, debug=False,
                   num_devices=C)
    f32, i16 = mybir.dt.float32, mybir.dt.int16
    bf16 = mybir.dt.bfloat16
    B = Builder(nc, mybir)

    x_arrs = []
    for c in range(C):
        xc = X[c::C]
        xc = np.vstack([xc, np.zeros((n0l - len(xc), D_IN), np.float32)])
        x_arrs.append(xc.astype(BF16))
    x_d = B.add_input("x", [n0l, D_IN], bf16, x_arrs)
    out_d = nc.dram_tensor("out", [n0l, D_OUT], f32, kind="ExternalOutput")

    def dram(name, rows, d, dt=bf16, shared=False):
        return nc.dram_tensor(name, [rows, d], dt,
                              addr_space="Shared" if shared else "Local")

    T0 = dram("T0", n0l, D_H)
    Y0p, Y0f = dram("Y0p", e0p, D_H), dram("Y0f", e0p, D_H, shared=True)
    h0 = dram("h0", n0l, D_H)
    P1p, P1s = dram("P1p", C * n1l, D_H), dram("P1s", n1l, D_H)
    T1 = dram("T1", n1l, D_H)
    Y1p, Y1f = dram("Y1p", e1p, D_H), dram("Y1f", e1p, D_H, shared=True)
    h1 = dram("h1", n1l, D_H)
    P2p, P2s = dram("P2p", C * n2l, D_H), dram("P2s", n2l, D_H)
    T2 = dram("T2", n2l, D_H)
    Y2p, Y2f = dram("Y2p", e2p, D_H), dram("Y2f", e2p, D_H, shared=True)
    Xc2, Xc2f = dram("Xc2", n2l, D_H), dram("Xc2f", C * n2l, D_H, shared=True)
    T3 = dram("T3", n1l, D_H)
    Y3p, Y3f = dram("Y3p", e1p, D_H), dram("Y3f", e1p, D_H, shared=True)
    Xu1, Xuf = dram("Xu1", n1l, D_H), dram("Xuf", C * n1l, D_H, shared=True)
    T4 = dram("T4", n0l, D_OUT, dt=f32)
    Y4p = dram("Y4p", e0p, D_OUT, dt=f32)
    Y4f = dram("Y4f", e0p, D_OUT, dt=f32, shared=True)

    up1_d = B.add_input("up1_idx", [128, up1_L // 16], i16, up1_planes)
    up0_d = B.add_input("up0_idx", [128, up0_L // 16], i16, up0_planes)

    rg = [list(range(C))]

    def AR(src, dst, nchunks=1, nw=None):
        if nchunks == 1:
            nc.gpsimd.collective_compute(
                "AllReduce", mybir.AluOpType.add, replica_groups=rg,
                ins=[src.ap().opt()], outs=[dst.ap().opt()])
            return
        bounds = [round(i * nw / nchunks) * 128 for i in range(nchunks + 1)]
        for a, b in zip(bounds, bounds[1:]):
            nc.gpsimd.collective_compute(
                "AllReduce", mybir.AluOpType.add, replica_groups=rg,
                ins=[src[a:b, :].opt()], outs=[dst[a:b, :].opt()])

    def RS(src, dst):
        nc.gpsimd.collective_compute(
            "ReduceScatter", mybir.AluOpType.add, replica_groups=rg,
            ins=[src.ap().opt()], outs=[dst.ap().opt()])

    def AG(src, dst):
        nc.gpsimd.collective_compute(
            "AllGather", mybir.AluOpType.bypass, replica_groups=rg,
            ins=[src.ap().opt()], outs=[dst.ap().opt()])

    with ExitStack() as ctx:
        tc = ctx.enter_context(tile.TileContext(nc))
        B.tc = tc
        B.setup_pools(ctx)
        B.setup_consts()
        W0t = B.const_mat("w0", np.asarray(inputs["W0"]), bf16)
        W1t = B.const_mat("w1m", np.asarray(inputs["W1"]), bf16)
        W2t = B.const_mat("w2m", np.asarray(inputs["W2"]), bf16)
        W3a = B.const_mat("w3a", np.asarray(inputs["W3"])[:128], bf16)
        W3b = B.const_mat("w3b", np.asarray(inputs["W3"])[128:], bf16)
        W4a = B.const_mat("w4a", np.asarray(inputs["W4"])[:128], bf16)
        W4b = B.const_mat("w4b", np.asarray(inputs["W4"])[128:], bf16)
        dv0 = B.add_input("dv0p", [128, n0l // 128], f32, dv_planes[0])
        dv1 = B.add_input("dv1p", [128, n1l // 128], f32, dv_planes[1])
        dv2 = B.add_input("dv2p", [128, n2l // 128], f32, dv_planes[2])
        de0 = B.add_input("de0p", [128, e0p // 128], f32, de_planes[0])
        de1 = B.add_input("de1p", [128, e1p // 128], f32, de_planes[1])
        de2 = B.add_input("de2p", [128, e2p // 128], f32, de_planes[2])
        ic1 = B.add_input("ic1p", [128, C * n1l // 128], f32, ic1p)
        ic2 = B.add_input("ic2p", [128, C * n2l // 128], f32, ic2p)

        def load_plane(d, cols, tag):
            t = B.p_const.tile([128, cols], f32, tag=tag)
            nc.sync.dma_start(t[:], d[:, :])
            return t

        dv0t = load_plane(dv0, n0l // 128, "dv0t")
        dv1t = load_plane(dv1, n1l // 128, "dv1t")
        dv2t = load_plane(dv2, n2l // 128, "dv2t")
        de0t = load_plane(de0, e0p // 128, "de0t")
        de1t = load_plane(de1, e1p // 128, "de1t")
        de2t = load_plane(de2, e2p // 128, "de2t")
        ic1t = load_plane(ic1, C * n1l // 128, "ic1t")
        ic2t = load_plane(ic2, C * n2l // 128, "ic2t")

        ins = inputs
        phases = [
            lambda: B.emit_linear("lin0", [(x_d, None)], [W0t], ins["b0"], T0,
                                  n0l // 128, D_H, scale_t=dv0t),
            lambda: B.emit_seg(st1_0, T0, Y0p, D_H, "copy", de0t),
            lambda: AR(Y0p, Y0f, 4, e0p // 128),
            lambda: B.emit_seg(st2_0, Y0f, h0, D_H, "relu", dv0t),
            lambda: B.emit_seg(pool0, h0, P1p, D_H, "copy", ic1t),
            lambda: RS(P1p, P1s),
            lambda: B.emit_linear("lin1", [(P1s, None)], [W1t], ins["b1"], T1,
                                  n1l // 128, D_H, scale_t=dv1t),
            lambda: B.emit_seg(st1_1, T1, Y1p, D_H, "copy", de1t),
            lambda: AR(Y1p, Y1f, 2, e1p // 128),
            lambda: B.emit_seg(st2_1, Y1f, h1, D_H, "relu", dv1t),
            lambda: B.emit_seg(pool1, h1, P2p, D_H, "copy", ic2t),
            lambda: RS(P2p, P2s),
            lambda: B.emit_linear("lin2", [(P2s, None)], [W2t], ins["b2"], T2,
                                  n2l // 128, D_H, scale_t=dv2t),
            lambda: B.emit_seg(st1_2, T2, Y2p, D_H, "copy", de2t),
            lambda: AR(Y2p, Y2f),
            lambda: B.emit_seg(st2_2, Y2f, Xc2, D_H, "relu", dv2t),
            lambda: AG(Xc2, Xc2f),
            lambda: B.emit_linear("lin3", [(Xc2f, up1_d), (h1, None)],
                                  [W3a, W3b], ins["b3"], T3, n1l // 128, D_H,
                                  scale_t=dv1t),
            lambda: B.emit_seg(st1_1.renamed("l1bs1"), T3, Y3p, D_H, "copy",
                               de1t),
            lambda: AR(Y3p, Y3f, 2, e1p // 128),
            lambda: B.emit_seg(st2_1.renamed("l1bs2"), Y3f, Xu1, D_H, "relu",
                               dv1t),
            lambda: AG(Xu1, Xuf),
            lambda: B.emit_linear("lin4", [(Xuf, up0_d), (h0, None)],
                                  [W4a, W4b], ins["b4"], T4, n0l // 128, D_OUT,
                                  scale_t=dv0t),
            lambda: B.emit_seg(st1_0.renamed("l0bs1"), T4, Y4p, D_OUT, "copy",
                               de0t, fp32=True),
            lambda: AR(Y4p, Y4f, 4, e0p // 128),
            lambda: B.emit_seg(st2_0.renamed("l0bs2"), Y4f, out_d, D_OUT,
                               "copy", dv0t, fp32=True),
        ]
        for ph in phases[:nphases]:
            ph()
    if do_compile:
        nc.compile()

    in_maps = []
    for c in range(C):
        m = {}
        for name, arrs in B.inputs.items():
            m[name] = arrs[c] if isinstance(arrs, list) else arrs
        in_maps.append(m)
    return nc, in_maps


LAST_EXEC_NS = None


def _install_ntff_hook():
    import contextlib, ctypes, os, types
    try:
        from antenv import axon_hooks  # noqa: F401
        return
    except ImportError:
        pass
    import antenv
    so_path = os.environ.get("PJRT_LIBRARY_PATH", "/opt/axon/libaxon_pjrt.so")
    try:
        lib = ctypes.CDLL(so_path)
    except OSError:
        lib = None
    hook = None
    if lib is not None and hasattr(lib, "axon_start_nrt_profile"):
        lib.axon_start_nrt_profile.argtypes = [
            ctypes.POINTER(ctypes.c_int64), ctypes.c_size_t]
        lib.axon_start_nrt_profile.restype = ctypes.c_int64
        lib.axon_stop_nrt_profile.argtypes = [ctypes.c_char_p]
        lib.axon_stop_nrt_profile.restype = ctypes.c_int64

        @contextlib.contextmanager
        def hook(output_dir, device_ids):
            import jax
            jax.devices()
            if device_ids:
                ids = (ctypes.c_int64 * len(device_ids))(*device_ids)
                rc = lib.axon_start_nrt_profile(ids, len(device_ids))
            else:
                rc = lib.axon_start_nrt_profile(None, 0)
            if rc != 0:
                raise RuntimeError(f"axon_start_nrt_profile rc={rc}")
            try:
                yield
            finally:
                lib.axon_stop_nrt_profile(str(output_dir).encode())

    mod = types.ModuleType("antenv.axon_hooks")
    mod._hook = hook
    mod.get_axon_ntff_profile_hook = lambda: mod._hook
    def _set(h):
        mod._hook = h
    mod.set_axon_ntff_profile_hook = _set
    sys.modules["antenv.axon_hooks"] = mod
    antenv.axon_hooks = mod


def kernel(**inputs):
    global LAST_EXEC_NS
    import os
    trace = os.environ.get("HGNN_TRACE", "0") == "1"
    if trace:
        _install_ntff_hook()
    nc, in_maps = build(inputs)
    from concourse.bass_utils import run_bass_kernel_spmd
    res = run_bass_kernel_spmd(nc, in_maps, core_ids=list(range(C)),
                               trace=trace)
    LAST_EXEC_NS = res.exec_time_ns
    out = np.empty((N0, D_OUT), np.float32)
    for c in range(C):
        n = len(range(c, N0, C))
        out[c::C] = res.results[c]["out"][:n]
    return out
